# revision 37
# baseline (speedup 1.0000x reference)
"""GQA attention block (dense_transformer) on 8 trn2 cores.

Sharding: tensor-parallel by kv-group. Core c owns kv-group c = 8 query
heads + 1 k + 1 v head (640 rows of W_qkv) and the matching 512 columns of
W_dense. hidden_states is replicated (passed transposed, bf16). Each core
returns a bf16 partial [4096, 2048] dense output; the host sums the 8
partials in f32.

v2 layout: per-head attention. PSUM = 5 qkv accumulators + cpsA/cpsB/aux
(8 banks total). Score chunks rotate through cpsA/cpsB/aux; PV accumulates
per 512-col half into new cpsA/cpsB instances whose groups stop exactly at
ki=3 / ki=7 where each half's softmax epilogue (reciprocal straight off the
PSUM ones-row, f32r ones-matmul broadcast in aux) runs and frees the bank.
Dense borrows the qkv slots; the v-transpose borrows aux. In phase C the
batch-1 scores additionally rotate through qkv4 (dense only needs qkv0..3),
giving a 4-deep score pipeline there. Emission is interleaved across
batches — qkv(b1) fills attention(b0)'s PE gaps and dense(b0) fills
attention(b1)'s — since each engine executes its queue in program order.
Bulk DMAs trigger from the Pool engine (SWDGE, alternating with SP) to keep
them off the shared HWDGE dispatcher; weight tiles load lazily at the hs
prefetch cadence so the first matmul unblocks early.
Simulated per-core time (TimelineSim cost model): 340 us vs 605 us for the
previous version; PE engine occupancy 93%.
"""
import numpy as np
import ml_dtypes
from contextlib import ExitStack

import bass_rust
import concourse.bass as bass
import concourse.mybir as mybir
from concourse import tile
from concourse.bass_utils import run_bass_kernel_spmd

dt = mybir.dt
bf16 = ml_dtypes.bfloat16

B, S, HID = 2, 1024, 4096
NKV, G, HD = 8, 8, 64
NPOS = B * S
INV = 0.125
NCORES = 8

# ---------------------------------------------------------------------------
# walrus in this container takes at most ONE sync-wait per instruction; Tile
# attaches several (tail drain especially). Split extras onto same-engine nops.
_orig_exit = tile.TileContext.__exit__


def _split_waits(nc):
    for bb in nc.m.functions[0].blocks:
        out, extra = [], 0
        for inst in bb.instructions:
            si = inst.sync_info
            if si is not None and len(si.on_wait) > 1:
                waits = list(si.on_wait)
                for w in waits[:-1]:
                    nop = mybir.InstNoOp(name=f"I-wsplit-{nc.next_id()}")
                    nop.engine = inst.engine
                    nop.sync_info = bass_rust.SyncInfo(on_wait=[w], on_update=[])
                    nc.register_instruction(nop, overwrite=True)
                    out.append(nop)
                    extra += 1
                inst.sync_info = bass_rust.SyncInfo(
                    on_wait=[waits[-1]], on_update=list(si.on_update)
                )
            out.append(inst)
        if extra:
            bb.instructions = out


def _patched_exit(self, exc_type, exc_val, exc_tb):
    r = _orig_exit(self, exc_type, exc_val, exc_tb)
    _split_waits(self.nc)
    return r


tile.TileContext.__exit__ = _patched_exit
# ---------------------------------------------------------------------------

_CACHED_NC = None


def _drive(*gens):
    live = list(gens)
    while live:
        for g in list(live):
            try:
                next(g)
            except StopIteration:
                live.remove(g)


def build_program():
    global _CACHED_NC
    if _CACHED_NC is not None:
        return _CACHED_NC
    nc = bass.Bass()
    hst_d = nc.declare_dram_parameter("hst", [32, 128, NPOS], dt.bfloat16, isOutput=False)
    wq_d = nc.declare_dram_parameter("wq", [32, 128, 640], dt.bfloat16, isOutput=False)
    wd_d = nc.declare_dram_parameter("wd", [4, 128, 4096], dt.bfloat16, isOutput=False)
    cst_d = nc.declare_dram_parameter("cst", [128, 2048], dt.bfloat16, isOutput=False)
    msk_d = nc.declare_dram_parameter("msk", [128, 128], dt.float32, isOutput=False)
    ab_d = nc.declare_dram_parameter("ab", [128, 128], dt.float32, isOutput=False)
    idn_d = nc.declare_dram_parameter("idn", [64, 64], dt.bfloat16, isOutput=False)
    outp_d = nc.declare_dram_parameter("outp", [32, 128, NPOS], dt.bfloat16, isOutput=True)

    AF = mybir.ActivationFunctionType
    # packed causal offsets for et: block ki has width 1024-128*ki
    koff = [0] * 8
    for ki in range(1, 8):
        koff[ki] = koff[ki - 1] + (1024 - 128 * (ki - 1))
    ET_W = koff[7] + (1024 - 128 * 7)  # 4608

    with ExitStack() as ctx:
        tc = ctx.enter_context(tile.TileContext(nc))
        cpool = ctx.enter_context(tc.tile_pool(name="const", bufs=1))
        wq_sb = [cpool.tile([128, 640], dt.bfloat16, tag=f"wq{k}", name=f"wq{k}")
                 for k in range(32)]
        nc.sync.dma_start(wq_sb[0][:], wq_d[0])  # first matmul unblocks asap
        cst_sb = cpool.tile([128, 2048], dt.bfloat16)
        msk_sb = cpool.tile([128, 128], dt.float32)
        ab_sb = cpool.tile([128, 128], dt.float32)
        idn_sb = cpool.tile([64, 64], dt.bfloat16)
        wd_sb = [cpool.tile([128, 4096], dt.bfloat16, tag=f"wd{kt}", name=f"wd{kt}")
                 for kt in range(4)]
        onesf = cpool.tile([1, 64], dt.float32)
        nc.vector.memset(onesf[:], 1.0)
        ones_r = cpool.tile([1, 64], dt.float32r)
        nc.scalar.copy(ones_r[:], onesf[:])

        hs_pool = ctx.enter_context(tc.tile_pool(name="hs", bufs=2))
        raw_pool = ctx.enter_context(tc.tile_pool(name="raw", bufs=2))
        tmp_pool = ctx.enter_context(tc.tile_pool(name="tmp", bufs=1))
        qp_pool = ctx.enter_context(tc.tile_pool(name="qp", bufs=2))
        kv_pool = ctx.enter_context(tc.tile_pool(name="kv", bufs=2))
        va_pool = ctx.enter_context(tc.tile_pool(name="va", bufs=2))
        et_pool = ctx.enter_context(tc.tile_pool(name="et", bufs=2))
        l_pool = ctx.enter_context(tc.tile_pool(name="l", bufs=1))
        rb_pool = ctx.enter_context(tc.tile_pool(name="rb", bufs=1))
        ctx_pool = ctx.enter_context(tc.tile_pool(name="ctx", bufs=2))
        dout_pool = ctx.enter_context(tc.tile_pool(name="dout", bufs=4))
        mm = ctx.enter_context(tc.tile_pool(name="mm", bufs=1, space="PSUM"))

        # per-batch SBUF state, filled by gen_qkv, read by gen_attn/gen_dense
        qp = {}   # (b, h) -> [64, 1024] bf16
        kk = {}   # b -> [64, 1024] bf16
        va = {}   # b -> [128, 8*72] bf16
        ctxt = {}  # (b, pr) -> [128, 1024] bf16

        def gen_qkv(b):
            for h in range(8):
                qp[(b, h)] = qp_pool.tile([64, 1024], dt.bfloat16, tag=f"qp{h}",
                                          name=f"qp{h}_{b}")
            kk[b] = kv_pool.tile([64, 1024], dt.bfloat16, tag="kk", name=f"kk{b}")
            vt = kv_pool.tile([64, 1024], dt.bfloat16, tag="vt", name=f"vt{b}")
            for n in range(2):
                pcol = b * 1024 + n * 512
                ncol = slice(n * 512, n * 512 + 512)
                ps = [mm.tile([128, 512], dt.float32, tag=f"qkv{m}", name=f"qkv{m}")
                      for m in range(5)]
                hs_t = {}

                def _load(k):
                    hs_t[k] = hs_pool.tile([128, 512], dt.bfloat16, tag=f"hs{k % 8}",
                                           name=f"hs{k}_{n}_{b}")
                    eng = nc.gpsimd if k % 2 == 0 else nc.sync
                    eng.dma_start(hs_t[k][:], hst_d[k][:, pcol:pcol + 512])
                    # lazy wq loads ride the same prefetch cadence on SP
                    if b == 0 and n == 0 and k > 0:
                        nc.sync.dma_start(wq_sb[k][:], wq_d[k])
                    if b == 0 and n == 0 and k == 8:
                        # small consts, needed from the RoPE/attention stages
                        nc.gpsimd.dma_start(cst_sb[:], cst_d[:])
                        nc.gpsimd.dma_start(msk_sb[:], msk_d[:])
                        nc.gpsimd.dma_start(ab_sb[:], ab_d[:])
                        nc.gpsimd.dma_start(idn_sb[:], idn_d[:])

                for k in range(8):
                    _load(k)
                raw = [raw_pool.tile([128, 512], dt.bfloat16, tag=f"raw{m}",
                                     name=f"raw{m}_{n}_{b}") for m in range(5)]
                for k in range(32):
                    if k + 8 < 32:
                        _load(k + 8)
                    for m in range(5):
                        nc.tensor.matmul(
                            ps[m][:],
                            wq_sb[k][:, m * 128:(m + 1) * 128],
                            hs_t[k][:],
                            start=(k == 0), stop=(k == 31),
                        )
                        if k == 31:
                            # drain each accumulator while PE finishes the rest
                            nc.scalar.copy(raw[m][:], ps[m][:])
                    if k % 2 == 1:
                        yield
                Cs = cst_sb[:, n * 512:(n + 1) * 512]
                Ss = cst_sb[:, 1024 + n * 512: 1024 + (n + 1) * 512]
                for grp in range(2):
                    A, Bb = raw[grp * 2], raw[grp * 2 + 1]
                    P1 = tmp_pool.tile([128, 512], dt.bfloat16, tag="P1")
                    P2 = tmp_pool.tile([128, 512], dt.bfloat16, tag="P2")
                    P3 = tmp_pool.tile([128, 512], dt.bfloat16, tag="P3")
                    P4 = tmp_pool.tile([128, 512], dt.bfloat16, tag="P4")
                    nc.vector.tensor_mul(P1[:], A[:], Cs)
                    nc.vector.tensor_mul(P2[:], Bb[:], Ss)
                    nc.vector.tensor_mul(P3[:], Bb[:], Cs)
                    nc.vector.tensor_mul(P4[:], A[:], Ss)
                    for i in range(4):
                        h = grp * 4 + i
                        sl = slice(32 * i, 32 * i + 32)
                        nc.vector.tensor_sub(qp[(b, h)][0:32, ncol], P1[sl, :], P2[sl, :])
                        nc.vector.tensor_add(qp[(b, h)][32:64, ncol], P3[sl, :], P4[sl, :])
                kvr = raw[4]
                pk1 = tmp_pool.tile([32, 512], dt.bfloat16, tag="pk1")
                pk2 = tmp_pool.tile([32, 512], dt.bfloat16, tag="pk2")
                pk3 = tmp_pool.tile([32, 512], dt.bfloat16, tag="pk3")
                pk4 = tmp_pool.tile([32, 512], dt.bfloat16, tag="pk4")
                nc.vector.tensor_mul(pk1[:], kvr[0:32, :], Cs[0:32, :])
                nc.vector.tensor_mul(pk2[:], kvr[32:64, :], Ss[32:64, :])
                nc.vector.tensor_mul(pk3[:], kvr[32:64, :], Cs[32:64, :])
                nc.vector.tensor_mul(pk4[:], kvr[0:32, :], Ss[0:32, :])
                nc.vector.tensor_sub(kk[b][0:32, ncol], pk1[:], pk2[:])
                nc.vector.tensor_add(kk[b][32:64, ncol], pk3[:], pk4[:])
                nc.vector.tensor_copy(vt[:, ncol], kvr[64:128, :])
                yield
            # V transpose + ones column (borrows the aux PSUM bank)
            va[b] = va_pool.tile([128, 8 * 72], dt.bfloat16, tag="va", name=f"va{b}")
            for ki in range(8):
                slot = mm.tile([128, 512], dt.float32, tag="aux", name=f"vps{ki}_{b}")
                vps = slot[:, 0:32].bitcast(dt.bfloat16)
                nc.tensor.transpose(vps, vt[0:64, ki * 128:(ki + 1) * 128],
                                    idn_sb[:, :])
                nc.vector.tensor_copy(va[b][:, ki * 72: ki * 72 + 64], vps)
                nc.vector.memset(va[b][:, ki * 72 + 64: ki * 72 + 65], 1.0)
            yield

        def gen_attn(b):
            if b == 0:
                for kt in range(4):  # wd needed from phase C; load during B
                    nc.sync.dma_start(wd_sb[kt][:], wd_d[kt])
            for pr in range(4):
                ctxt[(b, pr)] = ctx_pool.tile([128, 1024], dt.bfloat16,
                                              tag=f"ctxt{pr}", name=f"ctxt{pr}_{b}")
            # b=0 (phase B): qkv(1) owns all 5 qkv slots, so PV shares the
            # cpsA/cpsB score slots and runs after all 12 exps of the head.
            # b=1 (phase C): dense only uses qkv0/1, so PV accumulates in
            # qkv2/qkv3 and interleaves per-ki into the score stream.
            cps_tags = ("cpsA", "cpsB")
            pv_inline = False
            # phase C: dense only touches qkv0..3, so qkv4 is a free 4th
            # score slot; putting it first unblocks the b=1 head-0 chunk
            # before b=0's last epilogue releases cpsA/cpsB/aux
            rot = ("cpsA", "cpsB", "aux") if b == 0 else ("qkv4", "cpsA", "cpsB", "aux")
            for h in range(8):
                et = et_pool.tile([128, ET_W], dt.bfloat16, tag="et", name=f"et{h}_{b}")
                pr, hh = h // 2, h % 2
                rr = l_pool.tile([1, 1024], dt.float32r, tag="rr")
                rb = rb_pool.tile([64, 1024], dt.float32, tag="rb")
                cph = [mm.tile([128, 512], dt.float32, tag=cps_tags[0], name=f"cpA{h}_{b}"),
                       mm.tile([128, 512], dt.float32, tag=cps_tags[1], name=f"cpB{h}_{b}")]
                ci = 0

                def _pv(ki):
                    g0 = ki * 128
                    while g0 < 1024:
                        half = g0 // 512
                        g1 = min(1024, (half + 1) * 512)
                        loc = slice(g0 - half * 512, g1 - half * 512)
                        nc.tensor.matmul(
                            cph[half][0:65, loc],
                            va[b][:, ki * 72: ki * 72 + 65],
                            et[:, koff[ki] + g0 - ki * 128: koff[ki] + g1 - ki * 128],
                            start=(ki == 0), stop=(ki == (3 if half == 0 else 7)),
                            skip_group_check=True,
                        )
                        g0 = g1

                def _epi(ki):
                    # epilogue for the finished half: A after ki=3, B after 7
                    half = 0 if ki == 3 else 1
                    hs_ = slice(half * 512, half * 512 + 512)
                    # reciprocal straight from the PSUM ones-row
                    # (f32r is fp32-width; the gate only knows dtype != f32)
                    with nc.allow_low_precision(reason="f32r output is fp32-width"):
                        nc.vector.reciprocal(rr[0:1, hs_], cph[half][64:65, 0:512])
                    slot = mm.tile([128, 512], dt.float32, tag="aux",
                                   name=f"rps{h}{half}_{b}")
                    nc.tensor.matmul(slot[0:64, :], ones_r[:], rr[:, hs_],
                                     start=True, stop=True)
                    nc.vector.tensor_copy(rb[:, hs_], slot[0:64, :])
                    nc.vector.tensor_mul(
                        ctxt[(b, pr)][hh * 64:(hh + 1) * 64, hs_],
                        cph[half][0:64, 0:512], rb[:, hs_])

                for ki in range(8):
                    base = ki * 128
                    nchunks = (1024 - base + 511) // 512
                    for cj in range(nchunks):
                        c0 = base + cj * 512
                        cw = min(512, 1024 - c0)
                        sc = mm.tile([128, 512], dt.float32,
                                     tag=rot[ci % len(rot)],
                                     name=f"sc{h}{ki}{cj}_{b}")
                        ci += 1
                        nc.tensor.matmul(
                            sc[:, 0:cw],
                            kk[b][0:64, base:base + 128],
                            qp[(b, h)][0:64, c0:c0 + cw],
                            start=True, stop=True,
                        )
                        if cj == 0:
                            nc.vector.tensor_add(sc[:, 0:128], sc[:, 0:128], msk_sb[:])
                        abc = b * 64 + ki * 8 + h
                        nc.scalar.activation(
                            et[:, koff[ki] + (c0 - base): koff[ki] + (c0 - base) + cw],
                            sc[:, 0:cw], AF.Exp,
                            bias=ab_sb[:, abc:abc + 1], scale=INV)
                    if pv_inline:
                        _pv(ki)
                        if ki == 3 or ki == 7:
                            _epi(ki)
                            yield
                    if ki == 3:
                        yield
                if not pv_inline:
                    yield
                    for ki in range(8):
                        _pv(ki)
                        if ki == 3 or ki == 7:
                            _epi(ki)
                            yield

        def gen_dense(b):
            # b=1 runs after attention is done, so the score-rotation banks
            # are free for deeper accumulate/drain pipelining
            slots = ("qkv0", "qkv1", "qkv2", "qkv3") if b == 0 else (
                "qkv0", "qkv1", "qkv2", "qkv3", "aux", "cpsA")
            for mt in range(32):
                dsb = dout_pool.tile([128, 1024], dt.bfloat16, tag="dsb",
                                     name=f"dsb{mt}_{b}")
                for n2 in range(2):
                    dps = mm.tile([128, 512], dt.float32,
                                  tag=slots[(mt * 2 + n2) % len(slots)],
                                  name=f"d{mt}{n2}_{b}")
                    for kt in range(4):
                        nc.tensor.matmul(
                            dps[:],
                            wd_sb[kt][:, mt * 128:(mt + 1) * 128],
                            ctxt[(b, kt)][:, n2 * 512:(n2 + 1) * 512],
                            start=(kt == 0), stop=(kt == 3),
                        )
                    if n2 == 0:
                        nc.scalar.copy(dsb[:, 0:512], dps[:])
                    else:
                        nc.vector.tensor_copy(dsb[:, 512:1024], dps[:])
                nc.gpsimd.dma_start(
                    outp_d[mt][:, b * 1024: b * 1024 + 1024], dsb[:])
                yield

        _drive(gen_qkv(0))
        _drive(gen_attn(0), gen_qkv(1))
        _drive(gen_dense(0), gen_attn(1))
        _drive(gen_dense(1))

    _CACHED_NC = nc
    return nc


def host_prep(hidden_states, alibi, attention_mask, W_qkv, W_dense):
    hsT = np.ascontiguousarray(hidden_states.reshape(NPOS, HID).T).astype(bf16)
    hsT = hsT.reshape(32, 128, NPOS)

    j32 = np.arange(32)
    inv_freq = 1.0 / (10000.0 ** (2 * j32 / HD))
    t = np.arange(S, dtype=np.float64)
    fr = np.outer(inv_freq, t)                       # [32, S]
    cst = np.zeros((128, 2048), np.float32)
    cst[:, 0:1024] = np.tile(np.cos(fr), (4, 1))
    cst[:, 1024:2048] = np.tile(np.sin(fr), (4, 1))
    cst = cst.astype(bf16)

    # single causal diag block, [kpos, q] layout: masked where kpos > q
    mf = np.where(attention_mask[0, 0, 0:128, 0:128], -8e9, 0.0).astype(np.float32)
    msk = np.ascontiguousarray(mf.T)                 # [kpos, q]

    al = alibi.reshape(B, NKV * G, S) * INV          # [B, 64, S]

    perm = []
    for i in range(4):
        perm += [i * 64 + d for d in range(32)]
    for i in range(4):
        perm += [i * 64 + 32 + d for d in range(32)]
    for i in range(4, 8):
        perm += [i * 64 + d for d in range(32)]
    for i in range(4, 8):
        perm += [i * 64 + 32 + d for d in range(32)]
    perm += [512 + d for d in range(64)] + [576 + d for d in range(64)]
    perm = np.array(perm)

    idn = np.eye(64, dtype=np.float32).astype(bf16)
    in_maps = []
    for c in range(NCORES):
        Wg = W_qkv[c * 640:(c + 1) * 640][perm]       # [640, 4096]
        wq = np.ascontiguousarray(Wg.T).astype(bf16).reshape(32, 128, 640)
        Wd = W_dense[:, c * 512:(c + 1) * 512]        # [4096, 512]
        wd = np.ascontiguousarray(Wd.T).astype(bf16).reshape(4, 128, 4096)
        ab = np.zeros((128, 128), np.float32)
        for b in range(2):
            for ki in range(8):
                for h in range(8):
                    ab[:, b * 64 + ki * 8 + h] = al[b, c * 8 + h,
                                                    ki * 128:(ki + 1) * 128]
        in_maps.append({
            "hst": hsT, "wq": wq, "wd": wd, "cst": cst,
            "msk": msk, "ab": ab, "idn": idn,
        })
    return in_maps


def kernel(hidden_states, alibi, attention_mask, W_qkv, W_dense, _want_time=False):
    nc = build_program()
    in_maps = host_prep(np.asarray(hidden_states), np.asarray(alibi),
                        np.asarray(attention_mask), np.asarray(W_qkv),
                        np.asarray(W_dense))
    res = run_bass_kernel_spmd(nc, in_maps, list(range(NCORES)))
    acc = np.zeros((32, 128, NPOS), np.float32)
    for c in range(NCORES):
        acc += res.results[c]["outp"].astype(np.float32)
    out = acc.reshape(4096, NPOS).T.reshape(B, S, HID)
    if _want_time:
        return np.ascontiguousarray(out), res
    return np.ascontiguousarray(out)


# revision 39
# speedup vs baseline: 1.0211x; 1.0211x over previous
"""GQA attention block (dense_transformer) on 8 trn2 cores.

Sharding: tensor-parallel by kv-group. Core c owns kv-group c = 8 query
heads + 1 k + 1 v head (640 rows of W_qkv) and the matching 512 columns of
W_dense. hidden_states is replicated (passed transposed, bf16). Each core
returns a bf16 partial [4096, 2048] dense output; the host sums the 8
partials in f32.

v2 layout: per-head attention. PSUM = 5 qkv accumulators + cpsA/cpsB/aux
(8 banks total). Score chunks rotate through cpsA/cpsB/aux; PV accumulates
per 512-col half into new cpsA/cpsB instances whose groups stop exactly at
ki=3 / ki=7 where each half's softmax epilogue (reciprocal straight off the
PSUM ones-row, f32r ones-matmul broadcast in aux) runs and frees the bank.
Dense borrows the qkv slots; the v-transpose borrows aux. In phase C the
batch-1 scores additionally rotate through qkv4 (dense only needs qkv0..3),
giving a 4-deep score pipeline there. Emission is interleaved across
batches — qkv(b1) fills attention(b0)'s PE gaps and dense(b0) fills
attention(b1)'s — since each engine executes its queue in program order.
Bulk DMAs trigger from the Pool engine (SWDGE, alternating with SP) to keep
them off the shared HWDGE dispatcher; weight tiles load lazily at the hs
prefetch cadence so the first matmul unblocks early.
Simulated per-core time (TimelineSim cost model): 340 us vs 605 us for the
previous version; PE engine occupancy 93%.
"""
import numpy as np
import ml_dtypes
from contextlib import ExitStack

import bass_rust
import concourse.bass as bass
import concourse.mybir as mybir
from concourse import tile
from concourse.bass_utils import run_bass_kernel_spmd

dt = mybir.dt
bf16 = ml_dtypes.bfloat16

B, S, HID = 2, 1024, 4096
NKV, G, HD = 8, 8, 64
NPOS = B * S
INV = 0.125
NCORES = 8

# ---------------------------------------------------------------------------
# walrus in this container takes at most ONE sync-wait per instruction; Tile
# attaches several (tail drain especially). Split extras onto same-engine nops.
_orig_exit = tile.TileContext.__exit__


def _split_waits(nc):
    for bb in nc.m.functions[0].blocks:
        out, extra = [], 0
        for inst in bb.instructions:
            si = inst.sync_info
            if si is not None and len(si.on_wait) > 1:
                waits = list(si.on_wait)
                for w in waits[:-1]:
                    nop = mybir.InstNoOp(name=f"I-wsplit-{nc.next_id()}")
                    nop.engine = inst.engine
                    nop.sync_info = bass_rust.SyncInfo(on_wait=[w], on_update=[])
                    nc.register_instruction(nop, overwrite=True)
                    out.append(nop)
                    extra += 1
                inst.sync_info = bass_rust.SyncInfo(
                    on_wait=[waits[-1]], on_update=list(si.on_update)
                )
            out.append(inst)
        if extra:
            bb.instructions = out


def _patched_exit(self, exc_type, exc_val, exc_tb):
    r = _orig_exit(self, exc_type, exc_val, exc_tb)
    _split_waits(self.nc)
    return r


tile.TileContext.__exit__ = _patched_exit
# ---------------------------------------------------------------------------

_CACHED_NC = None


def _drive(*gens):
    live = list(gens)
    while live:
        for g in list(live):
            try:
                next(g)
            except StopIteration:
                live.remove(g)


def build_program():
    global _CACHED_NC
    if _CACHED_NC is not None:
        return _CACHED_NC
    nc = bass.Bass()
    hst_d = nc.declare_dram_parameter("hst", [32, 128, NPOS], dt.bfloat16, isOutput=False)
    wq_d = nc.declare_dram_parameter("wq", [32, 128, 640], dt.bfloat16, isOutput=False)
    wd_d = nc.declare_dram_parameter("wd", [4, 128, 4096], dt.bfloat16, isOutput=False)
    cst_d = nc.declare_dram_parameter("cst", [128, 2048], dt.bfloat16, isOutput=False)
    msk_d = nc.declare_dram_parameter("msk", [128, 128], dt.float32, isOutput=False)
    ab_d = nc.declare_dram_parameter("ab", [128, 128], dt.float32, isOutput=False)
    idn_d = nc.declare_dram_parameter("idn", [64, 64], dt.bfloat16, isOutput=False)
    outp_d = nc.declare_dram_parameter("outp", [32, 128, NPOS], dt.bfloat16, isOutput=True)

    AF = mybir.ActivationFunctionType
    # packed causal offsets for et: block ki has width 1024-128*ki
    koff = [0] * 8
    for ki in range(1, 8):
        koff[ki] = koff[ki - 1] + (1024 - 128 * (ki - 1))
    ET_W = koff[7] + (1024 - 128 * 7)  # 4608

    with ExitStack() as ctx:
        tc = ctx.enter_context(tile.TileContext(nc))
        cpool = ctx.enter_context(tc.tile_pool(name="const", bufs=1))
        wq_sb = [cpool.tile([128, 640], dt.bfloat16, tag=f"wq{k}", name=f"wq{k}")
                 for k in range(32)]
        nc.sync.dma_start(wq_sb[0][:], wq_d[0])  # first matmul unblocks asap
        cst_sb = cpool.tile([128, 2048], dt.bfloat16)
        msk_sb = cpool.tile([128, 128], dt.float32)
        ab_sb = cpool.tile([128, 128], dt.float32)
        idn_sb = cpool.tile([64, 64], dt.bfloat16)
        wd_sb = [cpool.tile([128, 4096], dt.bfloat16, tag=f"wd{kt}", name=f"wd{kt}")
                 for kt in range(4)]
        onesf = cpool.tile([1, 64], dt.float32)
        nc.vector.memset(onesf[:], 1.0)
        ones_r = cpool.tile([1, 64], dt.float32r)
        nc.scalar.copy(ones_r[:], onesf[:])

        hs_pool = ctx.enter_context(tc.tile_pool(name="hs", bufs=2))
        raw_pool = ctx.enter_context(tc.tile_pool(name="raw", bufs=2))
        tmp_pool = ctx.enter_context(tc.tile_pool(name="tmp", bufs=1))
        qp_pool = ctx.enter_context(tc.tile_pool(name="qp", bufs=2))
        kv_pool = ctx.enter_context(tc.tile_pool(name="kv", bufs=2))
        va_pool = ctx.enter_context(tc.tile_pool(name="va", bufs=2))
        et_pool = ctx.enter_context(tc.tile_pool(name="et", bufs=2))
        l_pool = ctx.enter_context(tc.tile_pool(name="l", bufs=1))
        rb_pool = ctx.enter_context(tc.tile_pool(name="rb", bufs=1))
        ctx_pool = ctx.enter_context(tc.tile_pool(name="ctx", bufs=2))
        dout_pool = ctx.enter_context(tc.tile_pool(name="dout", bufs=4))
        mm = ctx.enter_context(tc.tile_pool(name="mm", bufs=1, space="PSUM"))

        # per-batch SBUF state, filled by gen_qkv, read by gen_attn/gen_dense
        qp = {}   # (b, h) -> [64, 1024] bf16
        kk = {}   # b -> [64, 1024] bf16
        va = {}   # b -> [128, 8*72] bf16
        ctxt = {}  # (b, pr) -> [128, 1024] bf16

        def gen_qkv(b):
            for h in range(8):
                qp[(b, h)] = qp_pool.tile([64, 1024], dt.bfloat16, tag=f"qp{h}",
                                          name=f"qp{h}_{b}")
            kk[b] = kv_pool.tile([64, 1024], dt.bfloat16, tag="kk", name=f"kk{b}")
            vt = kv_pool.tile([64, 1024], dt.bfloat16, tag="vt", name=f"vt{b}")
            for n in range(2):
                pcol = b * 1024 + n * 512
                ncol = slice(n * 512, n * 512 + 512)
                ps = [mm.tile([128, 512], dt.float32, tag=f"qkv{m}", name=f"qkv{m}")
                      for m in range(5)]
                hs_t = {}

                def _load(k):
                    hs_t[k] = hs_pool.tile([128, 512], dt.bfloat16, tag=f"hs{k % 8}",
                                           name=f"hs{k}_{n}_{b}")
                    eng = nc.gpsimd if k % 2 == 0 else nc.sync
                    eng.dma_start(hs_t[k][:], hst_d[k][:, pcol:pcol + 512])
                    # lazy wq loads ride the same prefetch cadence on SP
                    if b == 0 and n == 0 and k > 0:
                        nc.sync.dma_start(wq_sb[k][:], wq_d[k])
                    if b == 0 and n == 0 and k == 8:
                        # small consts, needed from the RoPE/attention stages
                        nc.gpsimd.dma_start(cst_sb[:], cst_d[:])
                        nc.gpsimd.dma_start(msk_sb[:], msk_d[:])
                        nc.gpsimd.dma_start(ab_sb[:], ab_d[:])
                        nc.gpsimd.dma_start(idn_sb[:], idn_d[:])

                for k in range(8):
                    _load(k)
                raw = [raw_pool.tile([128, 512], dt.bfloat16, tag=f"raw{m}",
                                     name=f"raw{m}_{n}_{b}") for m in range(5)]
                for k in range(32):
                    if k + 8 < 32:
                        _load(k + 8)
                    for m in range(5):
                        nc.tensor.matmul(
                            ps[m][:],
                            wq_sb[k][:, m * 128:(m + 1) * 128],
                            hs_t[k][:],
                            start=(k == 0), stop=(k == 31),
                        )
                        if k == 31:
                            # drain each accumulator while PE finishes the rest
                            nc.scalar.copy(raw[m][:], ps[m][:])
                    if k % 2 == 1:
                        yield
                Cs = cst_sb[:, n * 512:(n + 1) * 512]
                Ss = cst_sb[:, 1024 + n * 512: 1024 + (n + 1) * 512]
                for grp in range(2):
                    A, Bb = raw[grp * 2], raw[grp * 2 + 1]
                    P1 = tmp_pool.tile([128, 512], dt.bfloat16, tag="P1")
                    P2 = tmp_pool.tile([128, 512], dt.bfloat16, tag="P2")
                    P3 = tmp_pool.tile([128, 512], dt.bfloat16, tag="P3")
                    P4 = tmp_pool.tile([128, 512], dt.bfloat16, tag="P4")
                    nc.vector.tensor_mul(P1[:], A[:], Cs)
                    nc.vector.tensor_mul(P2[:], Bb[:], Ss)
                    nc.vector.tensor_mul(P3[:], Bb[:], Cs)
                    nc.vector.tensor_mul(P4[:], A[:], Ss)
                    for i in range(4):
                        h = grp * 4 + i
                        sl = slice(32 * i, 32 * i + 32)
                        nc.vector.tensor_sub(qp[(b, h)][0:32, ncol], P1[sl, :], P2[sl, :])
                        nc.vector.tensor_add(qp[(b, h)][32:64, ncol], P3[sl, :], P4[sl, :])
                kvr = raw[4]
                pk1 = tmp_pool.tile([32, 512], dt.bfloat16, tag="pk1")
                pk2 = tmp_pool.tile([32, 512], dt.bfloat16, tag="pk2")
                pk3 = tmp_pool.tile([32, 512], dt.bfloat16, tag="pk3")
                pk4 = tmp_pool.tile([32, 512], dt.bfloat16, tag="pk4")
                nc.vector.tensor_mul(pk1[:], kvr[0:32, :], Cs[0:32, :])
                nc.vector.tensor_mul(pk2[:], kvr[32:64, :], Ss[32:64, :])
                nc.vector.tensor_mul(pk3[:], kvr[32:64, :], Cs[32:64, :])
                nc.vector.tensor_mul(pk4[:], kvr[0:32, :], Ss[0:32, :])
                nc.vector.tensor_sub(kk[b][0:32, ncol], pk1[:], pk2[:])
                nc.vector.tensor_add(kk[b][32:64, ncol], pk3[:], pk4[:])
                nc.vector.tensor_copy(vt[:, ncol], kvr[64:128, :])
                yield
            # V transpose + ones column (borrows the aux PSUM bank)
            va[b] = va_pool.tile([128, 8 * 72], dt.bfloat16, tag="va", name=f"va{b}")
            for ki in range(8):
                slot = mm.tile([128, 512], dt.float32, tag="aux", name=f"vps{ki}_{b}")
                vps = slot[:, 0:32].bitcast(dt.bfloat16)
                nc.tensor.transpose(vps, vt[0:64, ki * 128:(ki + 1) * 128],
                                    idn_sb[:, :])
                nc.vector.tensor_copy(va[b][:, ki * 72: ki * 72 + 64], vps)
                nc.vector.memset(va[b][:, ki * 72 + 64: ki * 72 + 65], 1.0)
            yield

        def gen_attn(b):
            if b == 0:
                for kt in range(4):  # wd needed from phase C; load during B
                    nc.sync.dma_start(wd_sb[kt][:], wd_d[kt])
            for pr in range(4):
                ctxt[(b, pr)] = ctx_pool.tile([128, 1024], dt.bfloat16,
                                              tag=f"ctxt{pr}", name=f"ctxt{pr}_{b}")
            # b=0 (phase B): qkv(1) owns all 5 qkv slots, so PV shares the
            # cpsA/cpsB score slots and runs after all 12 exps of the head.
            # b=1 (phase C): dense only uses qkv0/1, so PV accumulates in
            # qkv2/qkv3 and interleaves per-ki into the score stream.
            cps_tags = ("cpsA", "cpsB")
            pv_inline = False
            # phase C: dense only touches qkv0..3, so qkv4 is a free 4th
            # score slot; putting it first unblocks the b=1 head-0 chunk
            # before b=0's last epilogue releases cpsA/cpsB/aux
            rot = ("cpsA", "aux", "cpsB") if b == 0 else ("qkv4", "cpsA", "aux", "cpsB")
            for h in range(8):
                et = et_pool.tile([128, ET_W], dt.bfloat16, tag="et", name=f"et{h}_{b}")
                pr, hh = h // 2, h % 2
                rr = l_pool.tile([1, 1024], dt.float32r, tag="rr")
                rb = rb_pool.tile([64, 1024], dt.float32, tag="rb")
                cph = [mm.tile([128, 512], dt.float32, tag=cps_tags[0], name=f"cpA{h}_{b}"),
                       mm.tile([128, 512], dt.float32, tag=cps_tags[1], name=f"cpB{h}_{b}")]
                ci = 0

                def _pv(ki):
                    g0 = ki * 128
                    while g0 < 1024:
                        half = g0 // 512
                        g1 = min(1024, (half + 1) * 512)
                        loc = slice(g0 - half * 512, g1 - half * 512)
                        nc.tensor.matmul(
                            cph[half][0:65, loc],
                            va[b][:, ki * 72: ki * 72 + 65],
                            et[:, koff[ki] + g0 - ki * 128: koff[ki] + g1 - ki * 128],
                            start=(ki == 0), stop=(ki == (3 if half == 0 else 7)),
                            skip_group_check=True,
                        )
                        g0 = g1

                def _epi(ki):
                    # epilogue for the finished half: A after ki=3, B after 7
                    half = 0 if ki == 3 else 1
                    hs_ = slice(half * 512, half * 512 + 512)
                    # reciprocal straight from the PSUM ones-row
                    # (f32r is fp32-width; the gate only knows dtype != f32)
                    with nc.allow_low_precision(reason="f32r output is fp32-width"):
                        nc.vector.reciprocal(rr[0:1, hs_], cph[half][64:65, 0:512])
                    slot = mm.tile([128, 512], dt.float32, tag="aux",
                                   name=f"rps{h}{half}_{b}")
                    nc.tensor.matmul(slot[0:64, :], ones_r[:], rr[:, hs_],
                                     start=True, stop=True)
                    nc.vector.tensor_copy(rb[:, hs_], slot[0:64, :])
                    nc.vector.tensor_mul(
                        ctxt[(b, pr)][hh * 64:(hh + 1) * 64, hs_],
                        cph[half][0:64, 0:512], rb[:, hs_])

                for ki in range(8):
                    base = ki * 128
                    nchunks = (1024 - base + 511) // 512
                    for cj in range(nchunks):
                        c0 = base + cj * 512
                        cw = min(512, 1024 - c0)
                        sc = mm.tile([128, 512], dt.float32,
                                     tag=rot[ci % len(rot)],
                                     name=f"sc{h}{ki}{cj}_{b}")
                        ci += 1
                        nc.tensor.matmul(
                            sc[:, 0:cw],
                            kk[b][0:64, base:base + 128],
                            qp[(b, h)][0:64, c0:c0 + cw],
                            start=True, stop=True,
                        )
                        if cj == 0:
                            nc.vector.tensor_add(sc[:, 0:128], sc[:, 0:128], msk_sb[:])
                        abc = b * 64 + ki * 8 + h
                        nc.scalar.activation(
                            et[:, koff[ki] + (c0 - base): koff[ki] + (c0 - base) + cw],
                            sc[:, 0:cw], AF.Exp,
                            bias=ab_sb[:, abc:abc + 1], scale=INV)
                    if pv_inline:
                        _pv(ki)
                        if ki == 3 or ki == 7:
                            _epi(ki)
                            yield
                    if ki == 3:
                        yield
                if not pv_inline:
                    yield
                    for ki in range(8):
                        _pv(ki)
                        if ki == 3 or ki == 7:
                            _epi(ki)
                            yield

        def gen_dense(b):
            # b=1 runs after attention is done, so the score-rotation banks
            # are free for deeper accumulate/drain pipelining
            slots = ("qkv0", "qkv1", "qkv2", "qkv3") if b == 0 else (
                "qkv0", "qkv1", "qkv2", "qkv3", "aux", "cpsA")
            for mt in range(32):
                dsb = dout_pool.tile([128, 1024], dt.bfloat16, tag="dsb",
                                     name=f"dsb{mt}_{b}")
                for n2 in range(2):
                    dps = mm.tile([128, 512], dt.float32,
                                  tag=slots[(mt * 2 + n2) % len(slots)],
                                  name=f"d{mt}{n2}_{b}")
                    for kt in range(4):
                        nc.tensor.matmul(
                            dps[:],
                            wd_sb[kt][:, mt * 128:(mt + 1) * 128],
                            ctxt[(b, kt)][:, n2 * 512:(n2 + 1) * 512],
                            start=(kt == 0), stop=(kt == 3),
                        )
                    if n2 == 0:
                        nc.scalar.copy(dsb[:, 0:512], dps[:])
                    else:
                        nc.vector.tensor_copy(dsb[:, 512:1024], dps[:])
                nc.gpsimd.dma_start(
                    outp_d[mt][:, b * 1024: b * 1024 + 1024], dsb[:])
                yield

        _drive(gen_qkv(0))
        _drive(gen_attn(0), gen_qkv(1))
        _drive(gen_attn(1), gen_dense(0))
        _drive(gen_dense(1))

    _CACHED_NC = nc
    return nc


def host_prep(hidden_states, alibi, attention_mask, W_qkv, W_dense):
    hsT = np.ascontiguousarray(hidden_states.reshape(NPOS, HID).T).astype(bf16)
    hsT = hsT.reshape(32, 128, NPOS)

    j32 = np.arange(32)
    inv_freq = 1.0 / (10000.0 ** (2 * j32 / HD))
    t = np.arange(S, dtype=np.float64)
    fr = np.outer(inv_freq, t)                       # [32, S]
    cst = np.zeros((128, 2048), np.float32)
    cst[:, 0:1024] = np.tile(np.cos(fr), (4, 1))
    cst[:, 1024:2048] = np.tile(np.sin(fr), (4, 1))
    cst = cst.astype(bf16)

    # single causal diag block, [kpos, q] layout: masked where kpos > q
    mf = np.where(attention_mask[0, 0, 0:128, 0:128], -8e9, 0.0).astype(np.float32)
    msk = np.ascontiguousarray(mf.T)                 # [kpos, q]

    al = alibi.reshape(B, NKV * G, S) * INV          # [B, 64, S]

    perm = []
    for i in range(4):
        perm += [i * 64 + d for d in range(32)]
    for i in range(4):
        perm += [i * 64 + 32 + d for d in range(32)]
    for i in range(4, 8):
        perm += [i * 64 + d for d in range(32)]
    for i in range(4, 8):
        perm += [i * 64 + 32 + d for d in range(32)]
    perm += [512 + d for d in range(64)] + [576 + d for d in range(64)]
    perm = np.array(perm)

    idn = np.eye(64, dtype=np.float32).astype(bf16)
    in_maps = []
    for c in range(NCORES):
        Wg = W_qkv[c * 640:(c + 1) * 640][perm]       # [640, 4096]
        wq = np.ascontiguousarray(Wg.T).astype(bf16).reshape(32, 128, 640)
        Wd = W_dense[:, c * 512:(c + 1) * 512]        # [4096, 512]
        wd = np.ascontiguousarray(Wd.T).astype(bf16).reshape(4, 128, 4096)
        ab = np.zeros((128, 128), np.float32)
        for b in range(2):
            for ki in range(8):
                for h in range(8):
                    ab[:, b * 64 + ki * 8 + h] = al[b, c * 8 + h,
                                                    ki * 128:(ki + 1) * 128]
        in_maps.append({
            "hst": hsT, "wq": wq, "wd": wd, "cst": cst,
            "msk": msk, "ab": ab, "idn": idn,
        })
    return in_maps


def kernel(hidden_states, alibi, attention_mask, W_qkv, W_dense, _want_time=False):
    nc = build_program()
    in_maps = host_prep(np.asarray(hidden_states), np.asarray(alibi),
                        np.asarray(attention_mask), np.asarray(W_qkv),
                        np.asarray(W_dense))
    res = run_bass_kernel_spmd(nc, in_maps, list(range(NCORES)))
    acc = np.zeros((32, 128, NPOS), np.float32)
    for c in range(NCORES):
        acc += res.results[c]["outp"].astype(np.float32)
    out = acc.reshape(4096, NPOS).T.reshape(B, S, HID)
    if _want_time:
        return np.ascontiguousarray(out), res
    return np.ascontiguousarray(out)


# revision 40
# speedup vs baseline: 1.0285x; 1.0072x over previous
"""GQA attention block (dense_transformer) on 8 trn2 cores.

Sharding: tensor-parallel by kv-group. Core c owns kv-group c = 8 query
heads + 1 k + 1 v head (640 rows of W_qkv) and the matching 512 columns of
W_dense. hidden_states is replicated (passed transposed, bf16). Each core
returns a bf16 partial [4096, 2048] dense output; the host sums the 8
partials in f32.

v2 layout: per-head attention. PSUM = 5 qkv accumulators + cpsA/cpsB/aux
(8 banks total). Score chunks rotate through cpsA/cpsB/aux; PV accumulates
per 512-col half into new cpsA/cpsB instances whose groups stop exactly at
ki=3 / ki=7 where each half's softmax epilogue (reciprocal straight off the
PSUM ones-row, f32r ones-matmul broadcast in aux) runs and frees the bank.
Dense borrows the qkv slots; the v-transpose borrows aux. In phase C the
batch-1 scores additionally rotate through qkv4 (dense only needs qkv0..3),
giving a 4-deep score pipeline there. Emission is interleaved across
batches — qkv(b1) fills attention(b0)'s PE gaps and dense(b0) fills
attention(b1)'s — since each engine executes its queue in program order.
Bulk DMAs trigger from the Pool engine (SWDGE, alternating with SP) to keep
them off the shared HWDGE dispatcher; weight tiles load lazily at the hs
prefetch cadence so the first matmul unblocks early.
Simulated per-core time (TimelineSim cost model): 340 us vs 605 us for the
previous version; PE engine occupancy 93%.
"""
import numpy as np
import ml_dtypes
from contextlib import ExitStack

import bass_rust
import concourse.bass as bass
import concourse.mybir as mybir
from concourse import tile
from concourse.bass_utils import run_bass_kernel_spmd

dt = mybir.dt
bf16 = ml_dtypes.bfloat16

B, S, HID = 2, 1024, 4096
NKV, G, HD = 8, 8, 64
NPOS = B * S
INV = 0.125
NCORES = 8

# ---------------------------------------------------------------------------
# walrus in this container takes at most ONE sync-wait per instruction; Tile
# attaches several (tail drain especially). Split extras onto same-engine nops.
_orig_exit = tile.TileContext.__exit__


def _split_waits(nc):
    for bb in nc.m.functions[0].blocks:
        out, extra = [], 0
        for inst in bb.instructions:
            si = inst.sync_info
            if si is not None and len(si.on_wait) > 1:
                waits = list(si.on_wait)
                for w in waits[:-1]:
                    nop = mybir.InstNoOp(name=f"I-wsplit-{nc.next_id()}")
                    nop.engine = inst.engine
                    nop.sync_info = bass_rust.SyncInfo(on_wait=[w], on_update=[])
                    nc.register_instruction(nop, overwrite=True)
                    out.append(nop)
                    extra += 1
                inst.sync_info = bass_rust.SyncInfo(
                    on_wait=[waits[-1]], on_update=list(si.on_update)
                )
            out.append(inst)
        if extra:
            bb.instructions = out


def _patched_exit(self, exc_type, exc_val, exc_tb):
    r = _orig_exit(self, exc_type, exc_val, exc_tb)
    _split_waits(self.nc)
    return r


tile.TileContext.__exit__ = _patched_exit
# ---------------------------------------------------------------------------

_CACHED_NC = None


def _drive(*gens):
    live = list(gens)
    while live:
        for g in list(live):
            try:
                next(g)
            except StopIteration:
                live.remove(g)


def build_program():
    global _CACHED_NC
    if _CACHED_NC is not None:
        return _CACHED_NC
    nc = bass.Bass()
    hst_d = nc.declare_dram_parameter("hst", [32, 128, NPOS], dt.bfloat16, isOutput=False)
    wq_d = nc.declare_dram_parameter("wq", [32, 128, 640], dt.bfloat16, isOutput=False)
    wd_d = nc.declare_dram_parameter("wd", [4, 128, 4096], dt.bfloat16, isOutput=False)
    cst_d = nc.declare_dram_parameter("cst", [128, 2048], dt.bfloat16, isOutput=False)
    msk_d = nc.declare_dram_parameter("msk", [128, 128], dt.bfloat16, isOutput=False)
    ab_d = nc.declare_dram_parameter("ab", [128, 128], dt.float32, isOutput=False)
    idn_d = nc.declare_dram_parameter("idn", [64, 64], dt.bfloat16, isOutput=False)
    outp_d = nc.declare_dram_parameter("outp", [32, 128, NPOS], dt.bfloat16, isOutput=True)

    AF = mybir.ActivationFunctionType
    # packed causal offsets for et: block ki has width 1024-128*ki
    koff = [0] * 8
    for ki in range(1, 8):
        koff[ki] = koff[ki - 1] + (1024 - 128 * (ki - 1))
    ET_W = koff[7] + (1024 - 128 * 7)  # 4608

    with ExitStack() as ctx:
        tc = ctx.enter_context(tile.TileContext(nc))
        cpool = ctx.enter_context(tc.tile_pool(name="const", bufs=1))
        wq_sb = [cpool.tile([128, 640], dt.bfloat16, tag=f"wq{k}", name=f"wq{k}")
                 for k in range(32)]
        nc.sync.dma_start(wq_sb[0][:], wq_d[0])  # first matmul unblocks asap
        cst_sb = cpool.tile([128, 2048], dt.bfloat16)
        msk_sb = cpool.tile([128, 128], dt.bfloat16)
        ab_sb = cpool.tile([128, 128], dt.float32)
        idn_sb = cpool.tile([64, 64], dt.bfloat16)
        wd_sb = [cpool.tile([128, 4096], dt.bfloat16, tag=f"wd{kt}", name=f"wd{kt}")
                 for kt in range(4)]
        onesf = cpool.tile([1, 64], dt.float32)
        nc.vector.memset(onesf[:], 1.0)
        ones_r = cpool.tile([1, 64], dt.float32r)
        nc.scalar.copy(ones_r[:], onesf[:])

        hs_pool = ctx.enter_context(tc.tile_pool(name="hs", bufs=2))
        raw_pool = ctx.enter_context(tc.tile_pool(name="raw", bufs=2))
        tmp_pool = ctx.enter_context(tc.tile_pool(name="tmp", bufs=1))
        qp_pool = ctx.enter_context(tc.tile_pool(name="qp", bufs=2))
        kv_pool = ctx.enter_context(tc.tile_pool(name="kv", bufs=2))
        va_pool = ctx.enter_context(tc.tile_pool(name="va", bufs=2))
        et_pool = ctx.enter_context(tc.tile_pool(name="et", bufs=2))
        l_pool = ctx.enter_context(tc.tile_pool(name="l", bufs=1))
        rb_pool = ctx.enter_context(tc.tile_pool(name="rb", bufs=1))
        ctx_pool = ctx.enter_context(tc.tile_pool(name="ctx", bufs=2))
        dout_pool = ctx.enter_context(tc.tile_pool(name="dout", bufs=4))
        mm = ctx.enter_context(tc.tile_pool(name="mm", bufs=1, space="PSUM"))

        # per-batch SBUF state, filled by gen_qkv, read by gen_attn/gen_dense
        qp = {}   # (b, h) -> [64, 1024] bf16
        kk = {}   # b -> [64, 1024] bf16
        va = {}   # b -> [128, 8*72] bf16
        ctxt = {}  # (b, pr) -> [128, 1024] bf16

        def gen_qkv(b):
            for h in range(8):
                qp[(b, h)] = qp_pool.tile([64, 1024], dt.bfloat16, tag=f"qp{h}",
                                          name=f"qp{h}_{b}")
            kk[b] = kv_pool.tile([64, 1024], dt.bfloat16, tag="kk", name=f"kk{b}")
            vt = kv_pool.tile([64, 1024], dt.bfloat16, tag="vt", name=f"vt{b}")
            for n in range(2):
                pcol = b * 1024 + n * 512
                ncol = slice(n * 512, n * 512 + 512)
                ps = [mm.tile([128, 512], dt.float32, tag=f"qkv{m}", name=f"qkv{m}")
                      for m in range(5)]
                hs_t = {}

                def _load(k):
                    hs_t[k] = hs_pool.tile([128, 512], dt.bfloat16, tag=f"hs{k % 8}",
                                           name=f"hs{k}_{n}_{b}")
                    eng = nc.gpsimd if k % 2 == 0 else nc.sync
                    eng.dma_start(hs_t[k][:], hst_d[k][:, pcol:pcol + 512])
                    # lazy wq loads ride the same prefetch cadence on SP
                    if b == 0 and n == 0 and k > 0:
                        nc.sync.dma_start(wq_sb[k][:], wq_d[k])
                    if b == 0 and n == 0 and k == 8:
                        # small consts, needed from the RoPE/attention stages
                        nc.gpsimd.dma_start(cst_sb[:], cst_d[:])
                        nc.gpsimd.dma_start(msk_sb[:], msk_d[:])
                        nc.gpsimd.dma_start(ab_sb[:], ab_d[:])
                        nc.gpsimd.dma_start(idn_sb[:], idn_d[:])

                for k in range(8):
                    _load(k)
                raw = [raw_pool.tile([128, 512], dt.bfloat16, tag=f"raw{m}",
                                     name=f"raw{m}_{n}_{b}") for m in range(5)]
                for k in range(32):
                    if k + 8 < 32:
                        _load(k + 8)
                    for m in range(5):
                        nc.tensor.matmul(
                            ps[m][:],
                            wq_sb[k][:, m * 128:(m + 1) * 128],
                            hs_t[k][:],
                            start=(k == 0), stop=(k == 31),
                        )
                        if k == 31:
                            # drain each accumulator while PE finishes the rest
                            nc.scalar.copy(raw[m][:], ps[m][:])
                    if k % 2 == 1:
                        yield
                Cs = cst_sb[:, n * 512:(n + 1) * 512]
                Ss = cst_sb[:, 1024 + n * 512: 1024 + (n + 1) * 512]
                for grp in range(2):
                    A, Bb = raw[grp * 2], raw[grp * 2 + 1]
                    P1 = tmp_pool.tile([128, 512], dt.bfloat16, tag="P1")
                    P2 = tmp_pool.tile([128, 512], dt.bfloat16, tag="P2")
                    P3 = tmp_pool.tile([128, 512], dt.bfloat16, tag="P3")
                    P4 = tmp_pool.tile([128, 512], dt.bfloat16, tag="P4")
                    nc.vector.tensor_mul(P1[:], A[:], Cs)
                    nc.vector.tensor_mul(P2[:], Bb[:], Ss)
                    nc.vector.tensor_mul(P3[:], Bb[:], Cs)
                    nc.vector.tensor_mul(P4[:], A[:], Ss)
                    for i in range(4):
                        h = grp * 4 + i
                        sl = slice(32 * i, 32 * i + 32)
                        nc.vector.tensor_sub(qp[(b, h)][0:32, ncol], P1[sl, :], P2[sl, :])
                        nc.vector.tensor_add(qp[(b, h)][32:64, ncol], P3[sl, :], P4[sl, :])
                kvr = raw[4]
                pk1 = tmp_pool.tile([32, 512], dt.bfloat16, tag="pk1")
                pk2 = tmp_pool.tile([32, 512], dt.bfloat16, tag="pk2")
                pk3 = tmp_pool.tile([32, 512], dt.bfloat16, tag="pk3")
                pk4 = tmp_pool.tile([32, 512], dt.bfloat16, tag="pk4")
                nc.vector.tensor_mul(pk1[:], kvr[0:32, :], Cs[0:32, :])
                nc.vector.tensor_mul(pk2[:], kvr[32:64, :], Ss[32:64, :])
                nc.vector.tensor_mul(pk3[:], kvr[32:64, :], Cs[32:64, :])
                nc.vector.tensor_mul(pk4[:], kvr[0:32, :], Ss[0:32, :])
                nc.vector.tensor_sub(kk[b][0:32, ncol], pk1[:], pk2[:])
                nc.vector.tensor_add(kk[b][32:64, ncol], pk3[:], pk4[:])
                nc.vector.tensor_copy(vt[:, ncol], kvr[64:128, :])
                yield
            # V transpose + ones column (borrows the aux PSUM bank)
            va[b] = va_pool.tile([128, 8 * 72], dt.bfloat16, tag="va", name=f"va{b}")
            for ki in range(8):
                slot = mm.tile([128, 512], dt.float32, tag="aux", name=f"vps{ki}_{b}")
                vps = slot[:, 0:32].bitcast(dt.bfloat16)
                nc.tensor.transpose(vps, vt[0:64, ki * 128:(ki + 1) * 128],
                                    idn_sb[:, :])
                nc.vector.tensor_copy(va[b][:, ki * 72: ki * 72 + 64], vps)
                nc.vector.memset(va[b][:, ki * 72 + 64: ki * 72 + 65], 1.0)
            yield

        def gen_attn(b):
            if b == 0:
                for kt in range(4):  # wd needed from phase C; load during B
                    nc.sync.dma_start(wd_sb[kt][:], wd_d[kt])
            for pr in range(4):
                ctxt[(b, pr)] = ctx_pool.tile([128, 1024], dt.bfloat16,
                                              tag=f"ctxt{pr}", name=f"ctxt{pr}_{b}")
            # b=0 (phase B): qkv(1) owns all 5 qkv slots, so PV shares the
            # cpsA/cpsB score slots and runs after all 12 exps of the head.
            # b=1 (phase C): dense only uses qkv0/1, so PV accumulates in
            # qkv2/qkv3 and interleaves per-ki into the score stream.
            cps_tags = ("cpsA", "cpsB")
            pv_inline = False
            # phase C: dense only touches qkv0..3, so qkv4 is a free 4th
            # score slot; putting it first unblocks the b=1 head-0 chunk
            # before b=0's last epilogue releases cpsA/cpsB/aux
            rot = ("cpsA", "aux", "cpsB") if b == 0 else ("qkv4", "cpsA", "aux", "cpsB")
            for h in range(8):
                et = et_pool.tile([128, ET_W], dt.bfloat16, tag="et", name=f"et{h}_{b}")
                pr, hh = h // 2, h % 2
                rr = l_pool.tile([1, 1024], dt.float32r, tag="rr")
                rb = rb_pool.tile([64, 1024], dt.float32, tag="rb")
                cph = [mm.tile([128, 512], dt.float32, tag=cps_tags[0], name=f"cpA{h}_{b}"),
                       mm.tile([128, 512], dt.float32, tag=cps_tags[1], name=f"cpB{h}_{b}")]
                ci = 0

                def _pv(ki):
                    g0 = ki * 128
                    while g0 < 1024:
                        half = g0 // 512
                        g1 = min(1024, (half + 1) * 512)
                        loc = slice(g0 - half * 512, g1 - half * 512)
                        nc.tensor.matmul(
                            cph[half][0:65, loc],
                            va[b][:, ki * 72: ki * 72 + 65],
                            et[:, koff[ki] + g0 - ki * 128: koff[ki] + g1 - ki * 128],
                            start=(ki == 0), stop=(ki == (3 if half == 0 else 7)),
                            skip_group_check=True,
                        )
                        g0 = g1

                def _epi(ki):
                    # epilogue for the finished half: A after ki=3, B after 7
                    half = 0 if ki == 3 else 1
                    hs_ = slice(half * 512, half * 512 + 512)
                    # reciprocal straight from the PSUM ones-row
                    # (f32r is fp32-width; the gate only knows dtype != f32)
                    with nc.allow_low_precision(reason="f32r output is fp32-width"):
                        nc.vector.reciprocal(rr[0:1, hs_], cph[half][64:65, 0:512])
                    slot = mm.tile([128, 512], dt.float32, tag="aux",
                                   name=f"rps{h}{half}_{b}")
                    nc.tensor.matmul(slot[0:64, :], ones_r[:], rr[:, hs_],
                                     start=True, stop=True)
                    nc.vector.tensor_copy(rb[:, hs_], slot[0:64, :])
                    nc.vector.tensor_mul(
                        ctxt[(b, pr)][hh * 64:(hh + 1) * 64, hs_],
                        cph[half][0:64, 0:512], rb[:, hs_])

                for ki in range(8):
                    base = ki * 128
                    nchunks = (1024 - base + 511) // 512
                    for cj in range(nchunks):
                        c0 = base + cj * 512
                        cw = min(512, 1024 - c0)
                        sc = mm.tile([128, 512], dt.float32,
                                     tag=rot[ci % len(rot)],
                                     name=f"sc{h}{ki}{cj}_{b}")
                        ci += 1
                        nc.tensor.matmul(
                            sc[:, 0:cw],
                            kk[b][0:64, base:base + 128],
                            qp[(b, h)][0:64, c0:c0 + cw],
                            start=True, stop=True,
                        )
                        abc = b * 64 + ki * 8 + h
                        nc.scalar.activation(
                            et[:, koff[ki] + (c0 - base): koff[ki] + (c0 - base) + cw],
                            sc[:, 0:cw], AF.Exp,
                            bias=ab_sb[:, abc:abc + 1], scale=INV)
                        if cj == 0:
                            # causal mask: zero the upper triangle of the diag
                            # block via a 0/1 multiply (off the sc->exp chain)
                            nc.vector.tensor_mul(
                                et[:, koff[ki]: koff[ki] + 128],
                                et[:, koff[ki]: koff[ki] + 128], msk_sb[:])
                    if pv_inline:
                        _pv(ki)
                        if ki == 3 or ki == 7:
                            _epi(ki)
                            yield
                    if ki == 3:
                        yield
                if not pv_inline:
                    yield
                    for ki in range(8):
                        _pv(ki)
                        if ki == 3 or ki == 7:
                            _epi(ki)
                            yield

        def gen_dense(b):
            # b=1 runs after attention is done, so the score-rotation banks
            # are free for deeper accumulate/drain pipelining
            slots = ("qkv0", "qkv1", "qkv2", "qkv3") if b == 0 else (
                "qkv0", "qkv1", "qkv2", "qkv3", "aux", "cpsA")
            for mt in range(32):
                dsb = dout_pool.tile([128, 1024], dt.bfloat16, tag="dsb",
                                     name=f"dsb{mt}_{b}")
                for n2 in range(2):
                    dps = mm.tile([128, 512], dt.float32,
                                  tag=slots[(mt * 2 + n2) % len(slots)],
                                  name=f"d{mt}{n2}_{b}")
                    for kt in range(4):
                        nc.tensor.matmul(
                            dps[:],
                            wd_sb[kt][:, mt * 128:(mt + 1) * 128],
                            ctxt[(b, kt)][:, n2 * 512:(n2 + 1) * 512],
                            start=(kt == 0), stop=(kt == 3),
                        )
                    if n2 == 0:
                        nc.scalar.copy(dsb[:, 0:512], dps[:])
                    else:
                        nc.vector.tensor_copy(dsb[:, 512:1024], dps[:])
                nc.gpsimd.dma_start(
                    outp_d[mt][:, b * 1024: b * 1024 + 1024], dsb[:])
                yield

        _drive(gen_qkv(0))
        _drive(gen_attn(0), gen_qkv(1))
        _drive(gen_attn(1), gen_dense(0))
        _drive(gen_dense(1))

    _CACHED_NC = nc
    return nc


def host_prep(hidden_states, alibi, attention_mask, W_qkv, W_dense):
    hsT = np.ascontiguousarray(hidden_states.reshape(NPOS, HID).T).astype(bf16)
    hsT = hsT.reshape(32, 128, NPOS)

    j32 = np.arange(32)
    inv_freq = 1.0 / (10000.0 ** (2 * j32 / HD))
    t = np.arange(S, dtype=np.float64)
    fr = np.outer(inv_freq, t)                       # [32, S]
    cst = np.zeros((128, 2048), np.float32)
    cst[:, 0:1024] = np.tile(np.cos(fr), (4, 1))
    cst[:, 1024:2048] = np.tile(np.sin(fr), (4, 1))
    cst = cst.astype(bf16)

    # single causal diag block, [kpos, q] layout: 0 where kpos > q, else 1
    mf = np.where(attention_mask[0, 0, 0:128, 0:128], 0.0, 1.0).astype(np.float32)
    msk = np.ascontiguousarray(mf.T).astype(bf16)    # [kpos, q]

    al = alibi.reshape(B, NKV * G, S) * INV          # [B, 64, S]

    perm = []
    for i in range(4):
        perm += [i * 64 + d for d in range(32)]
    for i in range(4):
        perm += [i * 64 + 32 + d for d in range(32)]
    for i in range(4, 8):
        perm += [i * 64 + d for d in range(32)]
    for i in range(4, 8):
        perm += [i * 64 + 32 + d for d in range(32)]
    perm += [512 + d for d in range(64)] + [576 + d for d in range(64)]
    perm = np.array(perm)

    idn = np.eye(64, dtype=np.float32).astype(bf16)
    in_maps = []
    for c in range(NCORES):
        Wg = W_qkv[c * 640:(c + 1) * 640][perm]       # [640, 4096]
        wq = np.ascontiguousarray(Wg.T).astype(bf16).reshape(32, 128, 640)
        Wd = W_dense[:, c * 512:(c + 1) * 512]        # [4096, 512]
        wd = np.ascontiguousarray(Wd.T).astype(bf16).reshape(4, 128, 4096)
        ab = np.zeros((128, 128), np.float32)
        for b in range(2):
            for ki in range(8):
                for h in range(8):
                    ab[:, b * 64 + ki * 8 + h] = al[b, c * 8 + h,
                                                    ki * 128:(ki + 1) * 128]
        in_maps.append({
            "hst": hsT, "wq": wq, "wd": wd, "cst": cst,
            "msk": msk, "ab": ab, "idn": idn,
        })
    return in_maps


def kernel(hidden_states, alibi, attention_mask, W_qkv, W_dense, _want_time=False):
    nc = build_program()
    in_maps = host_prep(np.asarray(hidden_states), np.asarray(alibi),
                        np.asarray(attention_mask), np.asarray(W_qkv),
                        np.asarray(W_dense))
    res = run_bass_kernel_spmd(nc, in_maps, list(range(NCORES)))
    acc = np.zeros((32, 128, NPOS), np.float32)
    for c in range(NCORES):
        acc += res.results[c]["outp"].astype(np.float32)
    out = acc.reshape(4096, NPOS).T.reshape(B, S, HID)
    if _want_time:
        return np.ascontiguousarray(out), res
    return np.ascontiguousarray(out)


# revision 43
# speedup vs baseline: 1.0311x; 1.0026x over previous
"""GQA attention block (dense_transformer) on 8 trn2 cores.

Sharding: tensor-parallel by kv-group. Core c owns kv-group c = 8 query
heads + 1 k + 1 v head (640 rows of W_qkv) and the matching 512 columns of
W_dense. hidden_states is replicated (passed transposed, bf16). Each core
returns a bf16 partial [4096, 2048] dense output; the host sums the 8
partials in f32.

v2 layout: per-head attention. PSUM = 5 qkv accumulators + cpsA/cpsB/aux
(8 banks total). Score chunks rotate through cpsA/cpsB/aux; PV accumulates
per 512-col half into new cpsA/cpsB instances whose groups stop exactly at
ki=3 / ki=7 where each half's softmax epilogue (reciprocal straight off the
PSUM ones-row, f32r ones-matmul broadcast in aux) runs and frees the bank.
Dense borrows the qkv slots; the v-transpose borrows aux. In phase C the
batch-1 scores additionally rotate through qkv4 (dense only needs qkv0..3),
giving a 4-deep score pipeline there. Emission is interleaved across
batches — qkv(b1) fills attention(b0)'s PE gaps and dense(b0) fills
attention(b1)'s — since each engine executes its queue in program order.
Bulk DMAs trigger from the Pool engine (SWDGE, alternating with SP) to keep
them off the shared HWDGE dispatcher; weight tiles load lazily at the hs
prefetch cadence so the first matmul unblocks early.
Simulated per-core time (TimelineSim cost model): 340 us vs 605 us for the
previous version; PE engine occupancy 93%.
"""
import numpy as np
import ml_dtypes
from contextlib import ExitStack

import bass_rust
import concourse.bass as bass
import concourse.mybir as mybir
from concourse import tile
from concourse.bass_utils import run_bass_kernel_spmd

dt = mybir.dt
bf16 = ml_dtypes.bfloat16

B, S, HID = 2, 1024, 4096
NKV, G, HD = 8, 8, 64
NPOS = B * S
INV = 0.125
NCORES = 8

# ---------------------------------------------------------------------------
# walrus in this container takes at most ONE sync-wait per instruction; Tile
# attaches several (tail drain especially). Split extras onto same-engine nops.
_orig_exit = tile.TileContext.__exit__


def _split_waits(nc):
    for bb in nc.m.functions[0].blocks:
        out, extra = [], 0
        for inst in bb.instructions:
            si = inst.sync_info
            if si is not None and len(si.on_wait) > 1:
                waits = list(si.on_wait)
                for w in waits[:-1]:
                    nop = mybir.InstNoOp(name=f"I-wsplit-{nc.next_id()}")
                    nop.engine = inst.engine
                    nop.sync_info = bass_rust.SyncInfo(on_wait=[w], on_update=[])
                    nc.register_instruction(nop, overwrite=True)
                    out.append(nop)
                    extra += 1
                inst.sync_info = bass_rust.SyncInfo(
                    on_wait=[waits[-1]], on_update=list(si.on_update)
                )
            out.append(inst)
        if extra:
            bb.instructions = out


def _patched_exit(self, exc_type, exc_val, exc_tb):
    r = _orig_exit(self, exc_type, exc_val, exc_tb)
    _split_waits(self.nc)
    return r


tile.TileContext.__exit__ = _patched_exit
# ---------------------------------------------------------------------------

_CACHED_NC = None


def _drive(*gens):
    live = list(gens)
    while live:
        for g in list(live):
            try:
                next(g)
            except StopIteration:
                live.remove(g)


def build_program():
    global _CACHED_NC
    if _CACHED_NC is not None:
        return _CACHED_NC
    nc = bass.Bass()
    hst_d = nc.declare_dram_parameter("hst", [32, 128, NPOS], dt.bfloat16, isOutput=False)
    wq_d = nc.declare_dram_parameter("wq", [32, 128, 640], dt.bfloat16, isOutput=False)
    wd_d = nc.declare_dram_parameter("wd", [4, 128, 4096], dt.bfloat16, isOutput=False)
    cst_d = nc.declare_dram_parameter("cst", [128, 2048], dt.bfloat16, isOutput=False)
    msk_d = nc.declare_dram_parameter("msk", [128, 128], dt.bfloat16, isOutput=False)
    ab_d = nc.declare_dram_parameter("ab", [128, 128], dt.float32, isOutput=False)
    idn_d = nc.declare_dram_parameter("idn", [64, 64], dt.bfloat16, isOutput=False)
    outp_d = nc.declare_dram_parameter("outp", [32, 128, NPOS], dt.bfloat16, isOutput=True)

    AF = mybir.ActivationFunctionType
    # packed causal offsets for et: block ki has width 1024-128*ki
    koff = [0] * 8
    for ki in range(1, 8):
        koff[ki] = koff[ki - 1] + (1024 - 128 * (ki - 1))
    ET_W = koff[7] + (1024 - 128 * 7)  # 4608

    with ExitStack() as ctx:
        tc = ctx.enter_context(tile.TileContext(nc))
        cpool = ctx.enter_context(tc.tile_pool(name="const", bufs=1))
        wq_sb = [cpool.tile([128, 640], dt.bfloat16, tag=f"wq{k}", name=f"wq{k}")
                 for k in range(32)]
        nc.sync.dma_start(wq_sb[0][:], wq_d[0])  # first matmul unblocks asap
        cst_sb = cpool.tile([128, 2048], dt.bfloat16)
        msk_sb = cpool.tile([128, 128], dt.bfloat16)
        ab_sb = cpool.tile([128, 128], dt.float32)
        idn_sb = cpool.tile([64, 64], dt.bfloat16)
        wd_sb = [cpool.tile([128, 4096], dt.bfloat16, tag=f"wd{kt}", name=f"wd{kt}")
                 for kt in range(4)]
        onesf = cpool.tile([1, 64], dt.float32)
        nc.vector.memset(onesf[:], 1.0)
        ones_r = cpool.tile([1, 64], dt.float32r)
        nc.scalar.copy(ones_r[:], onesf[:])

        hs_pool = ctx.enter_context(tc.tile_pool(name="hs", bufs=2))
        raw_pool = ctx.enter_context(tc.tile_pool(name="raw", bufs=2))
        tmp_pool = ctx.enter_context(tc.tile_pool(name="tmp", bufs=1))
        qp_pool = ctx.enter_context(tc.tile_pool(name="qp", bufs=2))
        kv_pool = ctx.enter_context(tc.tile_pool(name="kv", bufs=2))
        va_pool = ctx.enter_context(tc.tile_pool(name="va", bufs=2))
        et_pool = ctx.enter_context(tc.tile_pool(name="et", bufs=2))
        l_pool = ctx.enter_context(tc.tile_pool(name="l", bufs=1))
        rb_pool = ctx.enter_context(tc.tile_pool(name="rb", bufs=1))
        ctx_pool = ctx.enter_context(tc.tile_pool(name="ctx", bufs=2))
        dout_pool = ctx.enter_context(tc.tile_pool(name="dout", bufs=4))
        mm = ctx.enter_context(tc.tile_pool(name="mm", bufs=1, space="PSUM"))

        # per-batch SBUF state, filled by gen_qkv, read by gen_attn/gen_dense
        qp = {}   # (b, h) -> [64, 1024] bf16
        kk = {}   # b -> [64, 1024] bf16
        va = {}   # b -> [128, 8*72] bf16
        ctxt = {}  # (b, pr) -> [128, 1024] bf16

        def gen_qkv(b):
            for h in range(8):
                qp[(b, h)] = qp_pool.tile([64, 1024], dt.bfloat16, tag=f"qp{h}",
                                          name=f"qp{h}_{b}")
            kk[b] = kv_pool.tile([64, 1024], dt.bfloat16, tag="kk", name=f"kk{b}")
            vt = kv_pool.tile([64, 1024], dt.bfloat16, tag="vt", name=f"vt{b}")
            for n in range(2):
                pcol = b * 1024 + n * 512
                ncol = slice(n * 512, n * 512 + 512)
                ps = [mm.tile([128, 512], dt.float32, tag=f"qkv{m}", name=f"qkv{m}")
                      for m in range(5)]
                hs_t = {}

                def _load(k):
                    hs_t[k] = hs_pool.tile([128, 512], dt.bfloat16, tag=f"hs{k % 8}",
                                           name=f"hs{k}_{n}_{b}")
                    eng = nc.gpsimd if k % 2 == 0 else nc.sync
                    eng.dma_start(hs_t[k][:], hst_d[k][:, pcol:pcol + 512])
                    # lazy wq loads ride the same prefetch cadence on SP
                    if b == 0 and n == 0 and k > 0:
                        nc.sync.dma_start(wq_sb[k][:], wq_d[k])
                    if b == 0 and n == 0 and k == 8:
                        # small consts, needed from the RoPE/attention stages
                        nc.gpsimd.dma_start(cst_sb[:], cst_d[:])
                        nc.gpsimd.dma_start(msk_sb[:], msk_d[:])
                        nc.gpsimd.dma_start(ab_sb[:], ab_d[:])
                        nc.gpsimd.dma_start(idn_sb[:], idn_d[:])

                for k in range(8):
                    _load(k)
                raw = [raw_pool.tile([128, 512], dt.bfloat16, tag=f"raw{m}",
                                     name=f"raw{m}_{n}_{b}") for m in range(5)]
                for k in range(32):
                    if k + 8 < 32:
                        _load(k + 8)
                    for m in range(5):
                        nc.tensor.matmul(
                            ps[m][:],
                            wq_sb[k][:, m * 128:(m + 1) * 128],
                            hs_t[k][:],
                            start=(k == 0), stop=(k == 31),
                        )
                        if k == 31:
                            # drain each accumulator while PE finishes the rest
                            nc.scalar.copy(raw[m][:], ps[m][:])
                    if k % 2 == 1:
                        yield
                Cs = cst_sb[:, n * 512:(n + 1) * 512]
                Ss = cst_sb[:, 1024 + n * 512: 1024 + (n + 1) * 512]
                for grp in range(2):
                    A, Bb = raw[grp * 2], raw[grp * 2 + 1]
                    P1 = tmp_pool.tile([128, 512], dt.bfloat16, tag="P1")
                    P2 = tmp_pool.tile([128, 512], dt.bfloat16, tag="P2")
                    P3 = tmp_pool.tile([128, 512], dt.bfloat16, tag="P3")
                    P4 = tmp_pool.tile([128, 512], dt.bfloat16, tag="P4")
                    nc.vector.tensor_mul(P1[:], A[:], Cs)
                    nc.vector.tensor_mul(P2[:], Bb[:], Ss)
                    nc.vector.tensor_mul(P3[:], Bb[:], Cs)
                    nc.vector.tensor_mul(P4[:], A[:], Ss)
                    for i in range(4):
                        h = grp * 4 + i
                        sl = slice(32 * i, 32 * i + 32)
                        nc.vector.tensor_sub(qp[(b, h)][0:32, ncol], P1[sl, :], P2[sl, :])
                        nc.vector.tensor_add(qp[(b, h)][32:64, ncol], P3[sl, :], P4[sl, :])
                kvr = raw[4]
                pk1 = tmp_pool.tile([32, 512], dt.bfloat16, tag="pk1")
                pk2 = tmp_pool.tile([32, 512], dt.bfloat16, tag="pk2")
                pk3 = tmp_pool.tile([32, 512], dt.bfloat16, tag="pk3")
                pk4 = tmp_pool.tile([32, 512], dt.bfloat16, tag="pk4")
                nc.vector.tensor_mul(pk1[:], kvr[0:32, :], Cs[0:32, :])
                nc.vector.tensor_mul(pk2[:], kvr[32:64, :], Ss[32:64, :])
                nc.vector.tensor_mul(pk3[:], kvr[32:64, :], Cs[32:64, :])
                nc.vector.tensor_mul(pk4[:], kvr[0:32, :], Ss[0:32, :])
                nc.vector.tensor_sub(kk[b][0:32, ncol], pk1[:], pk2[:])
                nc.vector.tensor_add(kk[b][32:64, ncol], pk3[:], pk4[:])
                nc.vector.tensor_copy(vt[:, ncol], kvr[64:128, :])
                yield
            # V transpose + ones column (borrows the aux PSUM bank)
            va[b] = va_pool.tile([128, 8 * 72], dt.bfloat16, tag="va", name=f"va{b}")
            for ki in range(8):
                slot = mm.tile([128, 512], dt.float32, tag="aux", name=f"vps{ki}_{b}")
                vps = slot[:, 0:32].bitcast(dt.bfloat16)
                nc.tensor.transpose(vps, vt[0:64, ki * 128:(ki + 1) * 128],
                                    idn_sb[:, :])
                nc.vector.tensor_copy(va[b][:, ki * 72: ki * 72 + 64], vps)
                nc.vector.memset(va[b][:, ki * 72 + 64: ki * 72 + 65], 1.0)
            yield

        def gen_attn(b):
            if b == 0:
                for kt in range(4):  # wd needed from phase C; load during B
                    nc.sync.dma_start(wd_sb[kt][:], wd_d[kt])
            for pr in range(4):
                ctxt[(b, pr)] = ctx_pool.tile([128, 1024], dt.bfloat16,
                                              tag=f"ctxt{pr}", name=f"ctxt{pr}_{b}")
            # b=0 (phase B): qkv(1) owns all 5 qkv slots, so PV shares the
            # cpsA/cpsB score slots and runs after all 12 exps of the head.
            # b=1 (phase C): dense only uses qkv0/1, so PV accumulates in
            # qkv2/qkv3 and interleaves per-ki into the score stream.
            cps_tags = ("cpsA", "cpsB")
            pv_inline = False
            # phase C: dense only touches qkv0..3, so qkv4 is a free 4th
            # score slot; putting it first unblocks the b=1 head-0 chunk
            # before b=0's last epilogue releases cpsA/cpsB/aux
            rot = ("cpsA", "aux", "cpsB") if b == 0 else ("qkv4", "cpsA", "aux", "cpsB")
            for h in range(8):
                et = et_pool.tile([128, ET_W], dt.bfloat16, tag="et", name=f"et{h}_{b}")
                pr, hh = h // 2, h % 2
                rr = l_pool.tile([1, 1024], dt.float32r, tag="rr")
                rb = rb_pool.tile([64, 1024], dt.float32, tag="rb")
                cph = [mm.tile([128, 512], dt.float32, tag=cps_tags[0], name=f"cpA{h}_{b}"),
                       mm.tile([128, 512], dt.float32, tag=cps_tags[1], name=f"cpB{h}_{b}")]
                ci = 0

                def _pv(ki):
                    g0 = ki * 128
                    while g0 < 1024:
                        half = g0 // 512
                        g1 = min(1024, (half + 1) * 512)
                        loc = slice(g0 - half * 512, g1 - half * 512)
                        nc.tensor.matmul(
                            cph[half][0:65, loc],
                            va[b][:, ki * 72: ki * 72 + 65],
                            et[:, koff[ki] + g0 - ki * 128: koff[ki] + g1 - ki * 128],
                            start=(ki == 0), stop=(ki == (3 if half == 0 else 7)),
                            skip_group_check=True,
                        )
                        g0 = g1

                def _epi(ki):
                    # epilogue for the finished half: A after ki=3, B after 7
                    half = 0 if ki == 3 else 1
                    hs_ = slice(half * 512, half * 512 + 512)
                    # reciprocal straight from the PSUM ones-row
                    # (f32r is fp32-width; the gate only knows dtype != f32)
                    with nc.allow_low_precision(reason="f32r output is fp32-width"):
                        nc.vector.reciprocal(rr[0:1, hs_], cph[half][64:65, 0:512])
                    slot = mm.tile([128, 512], dt.float32, tag="aux",
                                   name=f"rps{h}{half}_{b}")
                    nc.tensor.matmul(slot[0:64, :], ones_r[:], rr[:, hs_],
                                     start=True, stop=True)
                    nc.vector.tensor_copy(rb[:, hs_], slot[0:64, :])
                    nc.vector.tensor_mul(
                        ctxt[(b, pr)][hh * 64:(hh + 1) * 64, hs_],
                        cph[half][0:64, 0:512], rb[:, hs_])

                for ki in range(8):
                    base = ki * 128
                    nchunks = (1024 - base + 511) // 512
                    for cj in range(nchunks):
                        c0 = base + cj * 512
                        cw = min(512, 1024 - c0)
                        sc = mm.tile([128, 512], dt.float32,
                                     tag=rot[ci % len(rot)],
                                     name=f"sc{h}{ki}{cj}_{b}")
                        ci += 1
                        nc.tensor.matmul(
                            sc[:, 0:cw],
                            kk[b][0:64, base:base + 128],
                            qp[(b, h)][0:64, c0:c0 + cw],
                            start=True, stop=True,
                        )
                        abc = b * 64 + ki * 8 + h
                        nc.scalar.activation(
                            et[:, koff[ki] + (c0 - base): koff[ki] + (c0 - base) + cw],
                            sc[:, 0:cw], AF.Exp,
                            bias=ab_sb[:, abc:abc + 1], scale=INV)
                        if cj == 0:
                            # causal mask: zero the upper triangle of the diag
                            # block via a 0/1 multiply (off the sc->exp chain)
                            nc.vector.tensor_mul(
                                et[:, koff[ki]: koff[ki] + 128],
                                et[:, koff[ki]: koff[ki] + 128], msk_sb[:])
                    if pv_inline:
                        _pv(ki)
                        if ki == 3 or ki == 7:
                            _epi(ki)
                            yield
                    if ki == 3:
                        yield
                if not pv_inline:
                    yield
                    for ki in range(8):
                        _pv(ki)
                        if ki == 3 or ki == 7:
                            _epi(ki)
                            yield

        def gen_dense(b):
            # b=1 runs after attention is done, so the score-rotation banks
            # are free for deeper accumulate/drain pipelining
            slots = ("qkv0", "qkv1", "qkv2", "qkv3") if b == 0 else (
                "qkv0", "qkv1", "qkv2", "qkv3", "aux", "cpsA")
            for mt in range(32):
                dsb = dout_pool.tile([128, 1024], dt.bfloat16, tag="dsb",
                                     name=f"dsb{mt}_{b}")
                for n2 in range(2):
                    dps = mm.tile([128, 512], dt.float32,
                                  tag=slots[(mt * 2 + n2) % len(slots)],
                                  name=f"d{mt}{n2}_{b}")
                    for kt in range(4):
                        nc.tensor.matmul(
                            dps[:],
                            wd_sb[kt][:, mt * 128:(mt + 1) * 128],
                            ctxt[(b, kt)][:, n2 * 512:(n2 + 1) * 512],
                            start=(kt == 0), stop=(kt == 3),
                        )
                    if n2 == 0:
                        nc.scalar.copy(dsb[:, 0:512], dps[:])
                    else:
                        nc.vector.tensor_copy(dsb[:, 512:1024], dps[:])
                eng = nc.gpsimd if mt % 2 == 0 else nc.sync
                eng.dma_start(
                    outp_d[mt][:, b * 1024: b * 1024 + 1024], dsb[:])
                yield

        _drive(gen_qkv(0))
        _drive(gen_attn(0), gen_qkv(1))
        _drive(gen_attn(1), gen_dense(0))
        _drive(gen_dense(1))

    _CACHED_NC = nc
    return nc


def host_prep(hidden_states, alibi, attention_mask, W_qkv, W_dense):
    hsT = np.ascontiguousarray(hidden_states.reshape(NPOS, HID).T).astype(bf16)
    hsT = hsT.reshape(32, 128, NPOS)

    j32 = np.arange(32)
    inv_freq = 1.0 / (10000.0 ** (2 * j32 / HD))
    t = np.arange(S, dtype=np.float64)
    fr = np.outer(inv_freq, t)                       # [32, S]
    cst = np.zeros((128, 2048), np.float32)
    cst[:, 0:1024] = np.tile(np.cos(fr), (4, 1))
    cst[:, 1024:2048] = np.tile(np.sin(fr), (4, 1))
    cst = cst.astype(bf16)

    # single causal diag block, [kpos, q] layout: 0 where kpos > q, else 1
    mf = np.where(attention_mask[0, 0, 0:128, 0:128], 0.0, 1.0).astype(np.float32)
    msk = np.ascontiguousarray(mf.T).astype(bf16)    # [kpos, q]

    al = alibi.reshape(B, NKV * G, S) * INV          # [B, 64, S]

    perm = []
    for i in range(4):
        perm += [i * 64 + d for d in range(32)]
    for i in range(4):
        perm += [i * 64 + 32 + d for d in range(32)]
    for i in range(4, 8):
        perm += [i * 64 + d for d in range(32)]
    for i in range(4, 8):
        perm += [i * 64 + 32 + d for d in range(32)]
    perm += [512 + d for d in range(64)] + [576 + d for d in range(64)]
    perm = np.array(perm)

    idn = np.eye(64, dtype=np.float32).astype(bf16)
    in_maps = []
    for c in range(NCORES):
        Wg = W_qkv[c * 640:(c + 1) * 640][perm]       # [640, 4096]
        wq = np.ascontiguousarray(Wg.T).astype(bf16).reshape(32, 128, 640)
        Wd = W_dense[:, c * 512:(c + 1) * 512]        # [4096, 512]
        wd = np.ascontiguousarray(Wd.T).astype(bf16).reshape(4, 128, 4096)
        ab = np.zeros((128, 128), np.float32)
        for b in range(2):
            for ki in range(8):
                for h in range(8):
                    ab[:, b * 64 + ki * 8 + h] = al[b, c * 8 + h,
                                                    ki * 128:(ki + 1) * 128]
        in_maps.append({
            "hst": hsT, "wq": wq, "wd": wd, "cst": cst,
            "msk": msk, "ab": ab, "idn": idn,
        })
    return in_maps


def kernel(hidden_states, alibi, attention_mask, W_qkv, W_dense, _want_time=False):
    nc = build_program()
    in_maps = host_prep(np.asarray(hidden_states), np.asarray(alibi),
                        np.asarray(attention_mask), np.asarray(W_qkv),
                        np.asarray(W_dense))
    res = run_bass_kernel_spmd(nc, in_maps, list(range(NCORES)))
    acc = np.zeros((32, 128, NPOS), np.float32)
    for c in range(NCORES):
        acc += res.results[c]["outp"].astype(np.float32)
    out = acc.reshape(4096, NPOS).T.reshape(B, S, HID)
    if _want_time:
        return np.ascontiguousarray(out), res
    return np.ascontiguousarray(out)


# revision 45
# speedup vs baseline: 1.0312x; 1.0000x over previous
"""GQA attention block (dense_transformer) on 8 trn2 cores.

Sharding: tensor-parallel by kv-group. Core c owns kv-group c = 8 query
heads + 1 k + 1 v head (640 rows of W_qkv) and the matching 512 columns of
W_dense. hidden_states is replicated (passed transposed, bf16). Each core
returns a bf16 partial [4096, 2048] dense output; the host sums the 8
partials in f32.

v2 layout: per-head attention. PSUM = 5 qkv accumulators + cpsA/cpsB/aux
(8 banks total). Score chunks rotate through the spare slots ordered by
dependency depth (the slot whose previous epilogue frees last is used last);
in phase C batch-1 scores also rotate through qkv4 (dense needs qkv0..3
only), a 4-deep score pipeline. PV accumulates per 512-col half into new
cpsA/cpsB instances whose groups stop exactly at ki=3 / ki=7, where each
half's softmax epilogue (reciprocal straight off the PSUM ones-row, f32r
ones-matmul broadcast in aux) runs and frees the bank. The causal mask is a
0/1-triangle bf16 multiply on et AFTER the exp — off the score->exp chain
and out of PSUM. Dense borrows the qkv slots; the v-transpose borrows aux.
Emission is interleaved across batches — qkv(b1) fills attention(b0)'s PE
gaps and dense(b0) fills attention(b1)'s — since each engine executes its
queue in program order. Bulk DMAs trigger from the Pool engine (SWDGE,
alternating with SP) to keep them off the shared HWDGE dispatcher; weight
tiles load lazily at the hs prefetch cadence so the first matmul unblocks
early. Simulated per-core time (TimelineSim cost model): 330 us vs 605 us
for the previous version; PE engine occupancy 95%.
"""
import numpy as np
import ml_dtypes
from contextlib import ExitStack

import bass_rust
import concourse.bass as bass
import concourse.mybir as mybir
from concourse import tile
from concourse.bass_utils import run_bass_kernel_spmd

dt = mybir.dt
bf16 = ml_dtypes.bfloat16

B, S, HID = 2, 1024, 4096
NKV, G, HD = 8, 8, 64
NPOS = B * S
INV = 0.125
NCORES = 8

# ---------------------------------------------------------------------------
# walrus in this container takes at most ONE sync-wait per instruction; Tile
# attaches several (tail drain especially). Split extras onto same-engine nops.
_orig_exit = tile.TileContext.__exit__


def _split_waits(nc):
    for bb in nc.m.functions[0].blocks:
        out, extra = [], 0
        for inst in bb.instructions:
            si = inst.sync_info
            if si is not None and len(si.on_wait) > 1:
                waits = list(si.on_wait)
                for w in waits[:-1]:
                    nop = mybir.InstNoOp(name=f"I-wsplit-{nc.next_id()}")
                    nop.engine = inst.engine
                    nop.sync_info = bass_rust.SyncInfo(on_wait=[w], on_update=[])
                    nc.register_instruction(nop, overwrite=True)
                    out.append(nop)
                    extra += 1
                inst.sync_info = bass_rust.SyncInfo(
                    on_wait=[waits[-1]], on_update=list(si.on_update)
                )
            out.append(inst)
        if extra:
            bb.instructions = out


def _patched_exit(self, exc_type, exc_val, exc_tb):
    r = _orig_exit(self, exc_type, exc_val, exc_tb)
    _split_waits(self.nc)
    return r


tile.TileContext.__exit__ = _patched_exit
# ---------------------------------------------------------------------------

_CACHED_NC = None


def _drive(*gens):
    live = list(gens)
    while live:
        for g in list(live):
            try:
                next(g)
            except StopIteration:
                live.remove(g)


def build_program():
    global _CACHED_NC
    if _CACHED_NC is not None:
        return _CACHED_NC
    nc = bass.Bass()
    hst_d = nc.declare_dram_parameter("hst", [32, 128, NPOS], dt.bfloat16, isOutput=False)
    wq_d = nc.declare_dram_parameter("wq", [32, 128, 640], dt.bfloat16, isOutput=False)
    wd_d = nc.declare_dram_parameter("wd", [4, 128, 4096], dt.bfloat16, isOutput=False)
    cst_d = nc.declare_dram_parameter("cst", [128, 2048], dt.bfloat16, isOutput=False)
    msk_d = nc.declare_dram_parameter("msk", [128, 128], dt.bfloat16, isOutput=False)
    ab_d = nc.declare_dram_parameter("ab", [128, 128], dt.float32, isOutput=False)
    idn_d = nc.declare_dram_parameter("idn", [64, 64], dt.bfloat16, isOutput=False)
    outp_d = nc.declare_dram_parameter("outp", [32, 128, NPOS], dt.bfloat16, isOutput=True)

    AF = mybir.ActivationFunctionType
    # packed causal offsets for et: block ki has width 1024-128*ki
    koff = [0] * 8
    for ki in range(1, 8):
        koff[ki] = koff[ki - 1] + (1024 - 128 * (ki - 1))
    ET_W = koff[7] + (1024 - 128 * 7)  # 4608

    with ExitStack() as ctx:
        tc = ctx.enter_context(tile.TileContext(nc))
        cpool = ctx.enter_context(tc.tile_pool(name="const", bufs=1))
        wq_sb = [cpool.tile([128, 640], dt.bfloat16, tag=f"wq{k}", name=f"wq{k}")
                 for k in range(32)]
        nc.sync.dma_start(wq_sb[0][:], wq_d[0])  # first matmul unblocks asap
        cst_sb = cpool.tile([128, 2048], dt.bfloat16)
        msk_sb = cpool.tile([128, 128], dt.bfloat16)
        ab_sb = cpool.tile([128, 128], dt.float32)
        idn_sb = cpool.tile([64, 64], dt.bfloat16)
        wd_sb = [cpool.tile([128, 4096], dt.bfloat16, tag=f"wd{kt}", name=f"wd{kt}")
                 for kt in range(4)]
        onesf = cpool.tile([1, 64], dt.float32)
        nc.vector.memset(onesf[:], 1.0)
        ones_r = cpool.tile([1, 64], dt.float32r)
        nc.scalar.copy(ones_r[:], onesf[:])

        hs_pool = ctx.enter_context(tc.tile_pool(name="hs", bufs=2))
        raw_pool = ctx.enter_context(tc.tile_pool(name="raw", bufs=2))
        tmp_pool = ctx.enter_context(tc.tile_pool(name="tmp", bufs=1))
        qp_pool = ctx.enter_context(tc.tile_pool(name="qp", bufs=2))
        kv_pool = ctx.enter_context(tc.tile_pool(name="kv", bufs=2))
        va_pool = ctx.enter_context(tc.tile_pool(name="va", bufs=2))
        et_pool = ctx.enter_context(tc.tile_pool(name="et", bufs=2))
        l_pool = ctx.enter_context(tc.tile_pool(name="l", bufs=1))
        rb_pool = ctx.enter_context(tc.tile_pool(name="rb", bufs=1))
        ctx_pool = ctx.enter_context(tc.tile_pool(name="ctx", bufs=2))
        dout_pool = ctx.enter_context(tc.tile_pool(name="dout", bufs=4))
        mm = ctx.enter_context(tc.tile_pool(name="mm", bufs=1, space="PSUM"))

        # per-batch SBUF state, filled by gen_qkv, read by gen_attn/gen_dense
        qp = {}   # (b, h) -> [64, 1024] bf16
        kk = {}   # b -> [64, 1024] bf16
        va = {}   # b -> [128, 8*72] bf16
        ctxt = {}  # (b, pr) -> [128, 1024] bf16

        def gen_qkv(b):
            for h in range(8):
                qp[(b, h)] = qp_pool.tile([64, 1024], dt.bfloat16, tag=f"qp{h}",
                                          name=f"qp{h}_{b}")
            kk[b] = kv_pool.tile([64, 1024], dt.bfloat16, tag="kk", name=f"kk{b}")
            vt = kv_pool.tile([64, 1024], dt.bfloat16, tag="vt", name=f"vt{b}")
            for n in range(2):
                pcol = b * 1024 + n * 512
                ncol = slice(n * 512, n * 512 + 512)
                ps = [mm.tile([128, 512], dt.float32, tag=f"qkv{m}", name=f"qkv{m}")
                      for m in range(5)]
                hs_t = {}

                def _load(k):
                    hs_t[k] = hs_pool.tile([128, 512], dt.bfloat16, tag=f"hs{k % 8}",
                                           name=f"hs{k}_{n}_{b}")
                    eng = nc.gpsimd if k % 2 == 0 else nc.sync
                    eng.dma_start(hs_t[k][:], hst_d[k][:, pcol:pcol + 512])
                    # lazy wq loads ride the same prefetch cadence on SP
                    if b == 0 and n == 0 and k > 0:
                        nc.sync.dma_start(wq_sb[k][:], wq_d[k])
                    if b == 0 and n == 0 and k == 8:
                        # small consts, needed from the RoPE/attention stages
                        nc.gpsimd.dma_start(cst_sb[:], cst_d[:])
                        nc.gpsimd.dma_start(msk_sb[:], msk_d[:])
                        nc.gpsimd.dma_start(ab_sb[:], ab_d[:])
                        nc.gpsimd.dma_start(idn_sb[:], idn_d[:])

                for k in range(8):
                    _load(k)
                raw = [raw_pool.tile([128, 512], dt.bfloat16, tag=f"raw{m}",
                                     name=f"raw{m}_{n}_{b}") for m in range(5)]
                for k in range(32):
                    if k + 8 < 32:
                        _load(k + 8)
                    for m in range(5):
                        nc.tensor.matmul(
                            ps[m][:],
                            wq_sb[k][:, m * 128:(m + 1) * 128],
                            hs_t[k][:],
                            start=(k == 0), stop=(k == 31),
                        )
                        if k == 31:
                            # drain each accumulator while PE finishes the rest
                            nc.scalar.copy(raw[m][:], ps[m][:])
                    if k % 2 == 1:
                        yield
                Cs = cst_sb[:, n * 512:(n + 1) * 512]
                Ss = cst_sb[:, 1024 + n * 512: 1024 + (n + 1) * 512]
                for grp in range(2):
                    A, Bb = raw[grp * 2], raw[grp * 2 + 1]
                    P1 = tmp_pool.tile([128, 512], dt.bfloat16, tag="P1")
                    P2 = tmp_pool.tile([128, 512], dt.bfloat16, tag="P2")
                    P3 = tmp_pool.tile([128, 512], dt.bfloat16, tag="P3")
                    P4 = tmp_pool.tile([128, 512], dt.bfloat16, tag="P4")
                    nc.vector.tensor_mul(P1[:], A[:], Cs)
                    nc.vector.tensor_mul(P2[:], Bb[:], Ss)
                    nc.vector.tensor_mul(P3[:], Bb[:], Cs)
                    nc.vector.tensor_mul(P4[:], A[:], Ss)
                    for i in range(4):
                        h = grp * 4 + i
                        sl = slice(32 * i, 32 * i + 32)
                        nc.vector.tensor_sub(qp[(b, h)][0:32, ncol], P1[sl, :], P2[sl, :])
                        nc.vector.tensor_add(qp[(b, h)][32:64, ncol], P3[sl, :], P4[sl, :])
                kvr = raw[4]
                pk1 = tmp_pool.tile([32, 512], dt.bfloat16, tag="pk1")
                pk2 = tmp_pool.tile([32, 512], dt.bfloat16, tag="pk2")
                pk3 = tmp_pool.tile([32, 512], dt.bfloat16, tag="pk3")
                pk4 = tmp_pool.tile([32, 512], dt.bfloat16, tag="pk4")
                nc.vector.tensor_mul(pk1[:], kvr[0:32, :], Cs[0:32, :])
                nc.vector.tensor_mul(pk2[:], kvr[32:64, :], Ss[32:64, :])
                nc.vector.tensor_mul(pk3[:], kvr[32:64, :], Cs[32:64, :])
                nc.vector.tensor_mul(pk4[:], kvr[0:32, :], Ss[0:32, :])
                nc.vector.tensor_sub(kk[b][0:32, ncol], pk1[:], pk2[:])
                nc.vector.tensor_add(kk[b][32:64, ncol], pk3[:], pk4[:])
                nc.vector.tensor_copy(vt[:, ncol], kvr[64:128, :])
                yield
            # V transpose + ones column (borrows the aux PSUM bank)
            va[b] = va_pool.tile([128, 8 * 72], dt.bfloat16, tag="va", name=f"va{b}")
            for ki in range(8):
                slot = mm.tile([128, 512], dt.float32, tag="aux", name=f"vps{ki}_{b}")
                vps = slot[:, 0:32].bitcast(dt.bfloat16)
                nc.tensor.transpose(vps, vt[0:64, ki * 128:(ki + 1) * 128],
                                    idn_sb[:, :])
                nc.vector.tensor_copy(va[b][:, ki * 72: ki * 72 + 64], vps)
                nc.vector.memset(va[b][:, ki * 72 + 64: ki * 72 + 65], 1.0)
            yield

        def gen_attn(b):
            if b == 0:
                for kt in range(4):  # wd needed from phase C; load during B
                    nc.sync.dma_start(wd_sb[kt][:], wd_d[kt])
            for pr in range(4):
                ctxt[(b, pr)] = ctx_pool.tile([128, 1024], dt.bfloat16,
                                              tag=f"ctxt{pr}", name=f"ctxt{pr}_{b}")
            # b=0 (phase B): qkv(1) owns all 5 qkv slots, so PV shares the
            # cpsA/cpsB score slots and runs after all 12 exps of the head.
            # b=1 (phase C): dense only uses qkv0/1, so PV accumulates in
            # qkv2/qkv3 and interleaves per-ki into the score stream.
            cps_tags = ("cpsA", "cpsB")
            pv_inline = False
            # phase C: dense only touches qkv0..3, so qkv4 is a free 4th
            # score slot; putting it first unblocks the b=1 head-0 chunk
            # before b=0's last epilogue releases cpsA/cpsB/aux
            rot = ("cpsA", "aux", "cpsB") if b == 0 else ("cpsA", "qkv4", "aux", "cpsB")
            for h in range(8):
                et = et_pool.tile([128, ET_W], dt.bfloat16, tag="et", name=f"et{h}_{b}")
                pr, hh = h // 2, h % 2
                rr = l_pool.tile([1, 1024], dt.float32r, tag="rr")
                rb = rb_pool.tile([64, 1024], dt.float32, tag="rb")
                cph = [mm.tile([128, 512], dt.float32, tag=cps_tags[0], name=f"cpA{h}_{b}"),
                       mm.tile([128, 512], dt.float32, tag=cps_tags[1], name=f"cpB{h}_{b}")]
                ci = 0

                def _pv(ki):
                    g0 = ki * 128
                    while g0 < 1024:
                        half = g0 // 512
                        g1 = min(1024, (half + 1) * 512)
                        loc = slice(g0 - half * 512, g1 - half * 512)
                        nc.tensor.matmul(
                            cph[half][0:65, loc],
                            va[b][:, ki * 72: ki * 72 + 65],
                            et[:, koff[ki] + g0 - ki * 128: koff[ki] + g1 - ki * 128],
                            start=(ki == 0), stop=(ki == (3 if half == 0 else 7)),
                            skip_group_check=True,
                        )
                        g0 = g1

                def _epi(ki):
                    # epilogue for the finished half: A after ki=3, B after 7
                    half = 0 if ki == 3 else 1
                    hs_ = slice(half * 512, half * 512 + 512)
                    # reciprocal straight from the PSUM ones-row
                    # (f32r is fp32-width; the gate only knows dtype != f32)
                    with nc.allow_low_precision(reason="f32r output is fp32-width"):
                        nc.vector.reciprocal(rr[0:1, hs_], cph[half][64:65, 0:512])
                    slot = mm.tile([128, 512], dt.float32, tag="aux",
                                   name=f"rps{h}{half}_{b}")
                    nc.tensor.matmul(slot[0:64, :], ones_r[:], rr[:, hs_],
                                     start=True, stop=True)
                    nc.vector.tensor_copy(rb[:, hs_], slot[0:64, :])
                    nc.vector.tensor_mul(
                        ctxt[(b, pr)][hh * 64:(hh + 1) * 64, hs_],
                        cph[half][0:64, 0:512], rb[:, hs_])

                for ki in range(8):
                    base = ki * 128
                    nchunks = (1024 - base + 511) // 512
                    for cj in range(nchunks):
                        c0 = base + cj * 512
                        cw = min(512, 1024 - c0)
                        sc = mm.tile([128, 512], dt.float32,
                                     tag=rot[ci % len(rot)],
                                     name=f"sc{h}{ki}{cj}_{b}")
                        ci += 1
                        nc.tensor.matmul(
                            sc[:, 0:cw],
                            kk[b][0:64, base:base + 128],
                            qp[(b, h)][0:64, c0:c0 + cw],
                            start=True, stop=True,
                        )
                        abc = b * 64 + ki * 8 + h
                        nc.scalar.activation(
                            et[:, koff[ki] + (c0 - base): koff[ki] + (c0 - base) + cw],
                            sc[:, 0:cw], AF.Exp,
                            bias=ab_sb[:, abc:abc + 1], scale=INV)
                        if cj == 0:
                            # causal mask: zero the upper triangle of the diag
                            # block via a 0/1 multiply (off the sc->exp chain)
                            nc.vector.tensor_mul(
                                et[:, koff[ki]: koff[ki] + 128],
                                et[:, koff[ki]: koff[ki] + 128], msk_sb[:])
                    if pv_inline:
                        _pv(ki)
                        if ki == 3 or ki == 7:
                            _epi(ki)
                            yield
                    if ki == 3:
                        yield
                if not pv_inline:
                    yield
                    for ki in range(8):
                        _pv(ki)
                        if ki == 3 or ki == 7:
                            _epi(ki)
                            yield

        def gen_dense(b):
            # b=1 runs after attention is done, so the score-rotation banks
            # are free for deeper accumulate/drain pipelining
            slots = ("qkv0", "qkv1", "qkv2", "qkv3") if b == 0 else (
                "qkv0", "qkv1", "qkv2", "qkv3", "aux", "cpsA")
            for mt in range(32):
                dsb = dout_pool.tile([128, 1024], dt.bfloat16, tag="dsb",
                                     name=f"dsb{mt}_{b}")
                for n2 in range(2):
                    dps = mm.tile([128, 512], dt.float32,
                                  tag=slots[(mt * 2 + n2) % len(slots)],
                                  name=f"d{mt}{n2}_{b}")
                    for kt in range(4):
                        nc.tensor.matmul(
                            dps[:],
                            wd_sb[kt][:, mt * 128:(mt + 1) * 128],
                            ctxt[(b, kt)][:, n2 * 512:(n2 + 1) * 512],
                            start=(kt == 0), stop=(kt == 3),
                        )
                    if n2 == 0:
                        nc.scalar.copy(dsb[:, 0:512], dps[:])
                    else:
                        nc.vector.tensor_copy(dsb[:, 512:1024], dps[:])
                eng = nc.gpsimd if mt % 2 == 0 else nc.sync
                eng.dma_start(
                    outp_d[mt][:, b * 1024: b * 1024 + 1024], dsb[:])
                yield

        _drive(gen_qkv(0))
        _drive(gen_attn(0), gen_qkv(1))
        _drive(gen_attn(1), gen_dense(0))
        _drive(gen_dense(1))

    _CACHED_NC = nc
    return nc


def host_prep(hidden_states, alibi, attention_mask, W_qkv, W_dense):
    hsT = np.ascontiguousarray(hidden_states.reshape(NPOS, HID).T).astype(bf16)
    hsT = hsT.reshape(32, 128, NPOS)

    j32 = np.arange(32)
    inv_freq = 1.0 / (10000.0 ** (2 * j32 / HD))
    t = np.arange(S, dtype=np.float64)
    fr = np.outer(inv_freq, t)                       # [32, S]
    cst = np.zeros((128, 2048), np.float32)
    cst[:, 0:1024] = np.tile(np.cos(fr), (4, 1))
    cst[:, 1024:2048] = np.tile(np.sin(fr), (4, 1))
    cst = cst.astype(bf16)

    # single causal diag block, [kpos, q] layout: 0 where kpos > q, else 1
    mf = np.where(attention_mask[0, 0, 0:128, 0:128], 0.0, 1.0).astype(np.float32)
    msk = np.ascontiguousarray(mf.T).astype(bf16)    # [kpos, q]

    al = alibi.reshape(B, NKV * G, S) * INV          # [B, 64, S]

    perm = []
    for i in range(4):
        perm += [i * 64 + d for d in range(32)]
    for i in range(4):
        perm += [i * 64 + 32 + d for d in range(32)]
    for i in range(4, 8):
        perm += [i * 64 + d for d in range(32)]
    for i in range(4, 8):
        perm += [i * 64 + 32 + d for d in range(32)]
    perm += [512 + d for d in range(64)] + [576 + d for d in range(64)]
    perm = np.array(perm)

    idn = np.eye(64, dtype=np.float32).astype(bf16)
    in_maps = []
    for c in range(NCORES):
        Wg = W_qkv[c * 640:(c + 1) * 640][perm]       # [640, 4096]
        wq = np.ascontiguousarray(Wg.T).astype(bf16).reshape(32, 128, 640)
        Wd = W_dense[:, c * 512:(c + 1) * 512]        # [4096, 512]
        wd = np.ascontiguousarray(Wd.T).astype(bf16).reshape(4, 128, 4096)
        ab = np.zeros((128, 128), np.float32)
        for b in range(2):
            for ki in range(8):
                for h in range(8):
                    ab[:, b * 64 + ki * 8 + h] = al[b, c * 8 + h,
                                                    ki * 128:(ki + 1) * 128]
        in_maps.append({
            "hst": hsT, "wq": wq, "wd": wd, "cst": cst,
            "msk": msk, "ab": ab, "idn": idn,
        })
    return in_maps


def kernel(hidden_states, alibi, attention_mask, W_qkv, W_dense, _want_time=False):
    nc = build_program()
    in_maps = host_prep(np.asarray(hidden_states), np.asarray(alibi),
                        np.asarray(attention_mask), np.asarray(W_qkv),
                        np.asarray(W_dense))
    res = run_bass_kernel_spmd(nc, in_maps, list(range(NCORES)))
    acc = np.zeros((32, 128, NPOS), np.float32)
    for c in range(NCORES):
        acc += res.results[c]["outp"].astype(np.float32)
    out = acc.reshape(4096, NPOS).T.reshape(B, S, HID)
    if _want_time:
        return np.ascontiguousarray(out), res
    return np.ascontiguousarray(out)


# revision 46
# speedup vs baseline: 1.0325x; 1.0013x over previous
"""GQA attention block (dense_transformer) on 8 trn2 cores.

Sharding: tensor-parallel by kv-group. Core c owns kv-group c = 8 query
heads + 1 k + 1 v head (640 rows of W_qkv) and the matching 512 columns of
W_dense. hidden_states is replicated (passed transposed, bf16). Each core
returns a bf16 partial [4096, 2048] dense output; the host sums the 8
partials in f32.

v2 layout: per-head attention. PSUM = 5 qkv accumulators + cpsA/cpsB/aux
(8 banks total). Score chunks rotate through the spare slots ordered by
dependency depth (the slot whose previous epilogue frees last is used last);
in phase C batch-1 scores also rotate through qkv4 (dense needs qkv0..3
only), a 4-deep score pipeline. PV accumulates per 512-col half into new
cpsA/cpsB instances whose groups stop exactly at ki=3 / ki=7, where each
half's softmax epilogue (reciprocal straight off the PSUM ones-row, f32r
ones-matmul broadcast in aux) runs and frees the bank. The causal mask is a
0/1-triangle bf16 multiply on et AFTER the exp — off the score->exp chain
and out of PSUM. Dense borrows the qkv slots; the v-transpose borrows aux.
Emission is interleaved across batches — qkv(b1) fills attention(b0)'s PE
gaps and dense(b0) fills attention(b1)'s — since each engine executes its
queue in program order. Bulk DMAs trigger from the Pool engine (SWDGE,
alternating with SP) to keep them off the shared HWDGE dispatcher; weight
tiles load lazily at the hs prefetch cadence so the first matmul unblocks
early. Simulated per-core time (TimelineSim cost model): 330 us vs 605 us
for the previous version; PE engine occupancy 95%.
"""
import numpy as np
import ml_dtypes
from contextlib import ExitStack

import bass_rust
import concourse.bass as bass
import concourse.mybir as mybir
from concourse import tile
from concourse.bass_utils import run_bass_kernel_spmd

dt = mybir.dt
bf16 = ml_dtypes.bfloat16

B, S, HID = 2, 1024, 4096
NKV, G, HD = 8, 8, 64
NPOS = B * S
INV = 0.125
NCORES = 8

# ---------------------------------------------------------------------------
# walrus in this container takes at most ONE sync-wait per instruction; Tile
# attaches several (tail drain especially). Split extras onto same-engine nops.
_orig_exit = tile.TileContext.__exit__


def _split_waits(nc):
    for bb in nc.m.functions[0].blocks:
        out, extra = [], 0
        for inst in bb.instructions:
            si = inst.sync_info
            if si is not None and len(si.on_wait) > 1:
                waits = list(si.on_wait)
                for w in waits[:-1]:
                    nop = mybir.InstNoOp(name=f"I-wsplit-{nc.next_id()}")
                    nop.engine = inst.engine
                    nop.sync_info = bass_rust.SyncInfo(on_wait=[w], on_update=[])
                    nc.register_instruction(nop, overwrite=True)
                    out.append(nop)
                    extra += 1
                inst.sync_info = bass_rust.SyncInfo(
                    on_wait=[waits[-1]], on_update=list(si.on_update)
                )
            out.append(inst)
        if extra:
            bb.instructions = out


def _patched_exit(self, exc_type, exc_val, exc_tb):
    r = _orig_exit(self, exc_type, exc_val, exc_tb)
    _split_waits(self.nc)
    return r


tile.TileContext.__exit__ = _patched_exit
# ---------------------------------------------------------------------------

_CACHED_NC = None


def _drive(*gens):
    live = list(gens)
    while live:
        for g in list(live):
            try:
                next(g)
            except StopIteration:
                live.remove(g)


def build_program():
    global _CACHED_NC
    if _CACHED_NC is not None:
        return _CACHED_NC
    nc = bass.Bass()
    hst_d = nc.declare_dram_parameter("hst", [32, 128, NPOS], dt.bfloat16, isOutput=False)
    wq_d = nc.declare_dram_parameter("wq", [32, 128, 640], dt.bfloat16, isOutput=False)
    wd_d = nc.declare_dram_parameter("wd", [4, 128, 4096], dt.bfloat16, isOutput=False)
    cst_d = nc.declare_dram_parameter("cst", [128, 2048], dt.bfloat16, isOutput=False)
    msk_d = nc.declare_dram_parameter("msk", [128, 128], dt.bfloat16, isOutput=False)
    ab_d = nc.declare_dram_parameter("ab", [128, 128], dt.float32, isOutput=False)
    idn_d = nc.declare_dram_parameter("idn", [64, 64], dt.bfloat16, isOutput=False)
    outp_d = nc.declare_dram_parameter("outp", [32, 128, NPOS], dt.bfloat16, isOutput=True)

    AF = mybir.ActivationFunctionType
    # packed causal offsets for et: block ki has width 1024-128*ki
    koff = [0] * 8
    for ki in range(1, 8):
        koff[ki] = koff[ki - 1] + (1024 - 128 * (ki - 1))
    ET_W = koff[7] + (1024 - 128 * 7)  # 4608

    with ExitStack() as ctx:
        tc = ctx.enter_context(tile.TileContext(nc))
        cpool = ctx.enter_context(tc.tile_pool(name="const", bufs=1))
        wq_sb = [cpool.tile([128, 640], dt.bfloat16, tag=f"wq{k}", name=f"wq{k}")
                 for k in range(32)]
        nc.sync.dma_start(wq_sb[0][:], wq_d[0])  # first matmul unblocks asap
        cst_sb = cpool.tile([128, 2048], dt.bfloat16)
        msk_sb = cpool.tile([128, 128], dt.bfloat16)
        ab_sb = cpool.tile([128, 128], dt.float32)
        idn_sb = cpool.tile([64, 64], dt.bfloat16)
        wd_sb = [cpool.tile([128, 4096], dt.bfloat16, tag=f"wd{kt}", name=f"wd{kt}")
                 for kt in range(4)]
        onesf = cpool.tile([1, 64], dt.float32)
        nc.vector.memset(onesf[:], 1.0)
        ones_r = cpool.tile([1, 64], dt.float32r)
        nc.scalar.copy(ones_r[:], onesf[:])

        hs_pool = ctx.enter_context(tc.tile_pool(name="hs", bufs=2))
        raw_pool = ctx.enter_context(tc.tile_pool(name="raw", bufs=2))
        tmp_pool = ctx.enter_context(tc.tile_pool(name="tmp", bufs=1))
        qp_pool = ctx.enter_context(tc.tile_pool(name="qp", bufs=2))
        kv_pool = ctx.enter_context(tc.tile_pool(name="kv", bufs=2))
        va_pool = ctx.enter_context(tc.tile_pool(name="va", bufs=2))
        et_pool = ctx.enter_context(tc.tile_pool(name="et", bufs=2))
        l_pool = ctx.enter_context(tc.tile_pool(name="l", bufs=1))
        rb_pool = ctx.enter_context(tc.tile_pool(name="rb", bufs=1))
        ctx_pool = ctx.enter_context(tc.tile_pool(name="ctx", bufs=2))
        dout_pool = ctx.enter_context(tc.tile_pool(name="dout", bufs=4))
        mm = ctx.enter_context(tc.tile_pool(name="mm", bufs=1, space="PSUM"))

        # per-batch SBUF state, filled by gen_qkv, read by gen_attn/gen_dense
        qp = {}   # (b, h) -> [64, 1024] bf16
        kk = {}   # b -> [64, 1024] bf16
        va = {}   # b -> [128, 8*72] bf16
        ctxt = {}  # (b, pr) -> [128, 1024] bf16

        def gen_qkv(b):
            for h in range(8):
                qp[(b, h)] = qp_pool.tile([64, 1024], dt.bfloat16, tag=f"qp{h}",
                                          name=f"qp{h}_{b}")
            kk[b] = kv_pool.tile([64, 1024], dt.bfloat16, tag="kk", name=f"kk{b}")
            vt = kv_pool.tile([64, 1024], dt.bfloat16, tag="vt", name=f"vt{b}")
            for n in range(2):
                pcol = b * 1024 + n * 512
                ncol = slice(n * 512, n * 512 + 512)
                ps = [mm.tile([128, 512], dt.float32, tag=f"qkv{m}", name=f"qkv{m}")
                      for m in range(5)]
                hs_t = {}

                def _load(k):
                    hs_t[k] = hs_pool.tile([128, 512], dt.bfloat16, tag=f"hs{k % 8}",
                                           name=f"hs{k}_{n}_{b}")
                    eng = nc.gpsimd if k % 2 == 0 else nc.sync
                    eng.dma_start(hs_t[k][:], hst_d[k][:, pcol:pcol + 512])
                    # lazy wq loads ride the same prefetch cadence on SP
                    if b == 0 and n == 0 and k > 0:
                        nc.sync.dma_start(wq_sb[k][:], wq_d[k])
                    if b == 0 and n == 0 and k == 8:
                        # small consts, needed from the RoPE/attention stages
                        nc.gpsimd.dma_start(cst_sb[:], cst_d[:])
                        nc.gpsimd.dma_start(msk_sb[:], msk_d[:])
                        nc.gpsimd.dma_start(ab_sb[:], ab_d[:])
                        nc.gpsimd.dma_start(idn_sb[:], idn_d[:])

                for k in range(8):
                    _load(k)
                raw = [raw_pool.tile([128, 512], dt.bfloat16, tag=f"raw{m}",
                                     name=f"raw{m}_{n}_{b}") for m in range(5)]
                for k in range(32):
                    if k + 8 < 32:
                        _load(k + 8)
                    for m in range(5):
                        nc.tensor.matmul(
                            ps[m][:],
                            wq_sb[k][:, m * 128:(m + 1) * 128],
                            hs_t[k][:],
                            start=(k == 0), stop=(k == 31),
                        )
                        if k == 31:
                            # drain each accumulator while PE finishes the rest
                            nc.scalar.copy(raw[m][:], ps[m][:])
                    if k % 2 == 1:
                        yield
                Cs = cst_sb[:, n * 512:(n + 1) * 512]
                Ss = cst_sb[:, 1024 + n * 512: 1024 + (n + 1) * 512]
                for grp in range(2):
                    A, Bb = raw[grp * 2], raw[grp * 2 + 1]
                    P1 = tmp_pool.tile([128, 512], dt.bfloat16, tag="P1")
                    P2 = tmp_pool.tile([128, 512], dt.bfloat16, tag="P2")
                    P3 = tmp_pool.tile([128, 512], dt.bfloat16, tag="P3")
                    P4 = tmp_pool.tile([128, 512], dt.bfloat16, tag="P4")
                    nc.vector.tensor_mul(P1[:], A[:], Cs)
                    nc.vector.tensor_mul(P2[:], Bb[:], Ss)
                    nc.vector.tensor_mul(P3[:], Bb[:], Cs)
                    nc.vector.tensor_mul(P4[:], A[:], Ss)
                    for i in range(4):
                        h = grp * 4 + i
                        sl = slice(32 * i, 32 * i + 32)
                        nc.vector.tensor_sub(qp[(b, h)][0:32, ncol], P1[sl, :], P2[sl, :])
                        nc.vector.tensor_add(qp[(b, h)][32:64, ncol], P3[sl, :], P4[sl, :])
                kvr = raw[4]
                pk1 = tmp_pool.tile([32, 512], dt.bfloat16, tag="pk1")
                pk2 = tmp_pool.tile([32, 512], dt.bfloat16, tag="pk2")
                pk3 = tmp_pool.tile([32, 512], dt.bfloat16, tag="pk3")
                pk4 = tmp_pool.tile([32, 512], dt.bfloat16, tag="pk4")
                nc.vector.tensor_mul(pk1[:], kvr[0:32, :], Cs[0:32, :])
                nc.vector.tensor_mul(pk2[:], kvr[32:64, :], Ss[32:64, :])
                nc.vector.tensor_mul(pk3[:], kvr[32:64, :], Cs[32:64, :])
                nc.vector.tensor_mul(pk4[:], kvr[0:32, :], Ss[0:32, :])
                nc.vector.tensor_sub(kk[b][0:32, ncol], pk1[:], pk2[:])
                nc.vector.tensor_add(kk[b][32:64, ncol], pk3[:], pk4[:])
                nc.vector.tensor_copy(vt[:, ncol], kvr[64:128, :])
                yield
            # V transpose + ones column (borrows the aux PSUM bank)
            va[b] = va_pool.tile([128, 8 * 72], dt.bfloat16, tag="va", name=f"va{b}")
            for ki in range(8):
                slot = mm.tile([128, 512], dt.float32, tag="aux", name=f"vps{ki}_{b}")
                vps = slot[:, 0:32].bitcast(dt.bfloat16)
                nc.tensor.transpose(vps, vt[0:64, ki * 128:(ki + 1) * 128],
                                    idn_sb[:, :])
                nc.vector.tensor_copy(va[b][:, ki * 72: ki * 72 + 64], vps)
                nc.vector.memset(va[b][:, ki * 72 + 64: ki * 72 + 65], 1.0)
            yield

        def gen_attn(b):
            if b == 0:
                for kt in range(4):  # wd needed from phase C; load during B
                    nc.sync.dma_start(wd_sb[kt][:], wd_d[kt])
            for pr in range(4):
                ctxt[(b, pr)] = ctx_pool.tile([128, 1024], dt.bfloat16,
                                              tag=f"ctxt{pr}", name=f"ctxt{pr}_{b}")
            # b=0 (phase B): qkv(1) owns all 5 qkv slots, so PV shares the
            # cpsA/cpsB score slots and runs after all 12 exps of the head.
            # b=1 (phase C): dense only uses qkv0/1, so PV accumulates in
            # qkv2/qkv3 and interleaves per-ki into the score stream.
            cps_tags = ("cpsA", "cpsB")
            pv_inline = False
            # phase C: dense only touches qkv0..3, so qkv4 is a free 4th
            # score slot; putting it first unblocks the b=1 head-0 chunk
            # before b=0's last epilogue releases cpsA/cpsB/aux
            rot = ("cpsA", "aux", "cpsB") if b == 0 else ("cpsA", "qkv4", "aux", "cpsB")
            for h in range(8):
                et = et_pool.tile([128, ET_W], dt.bfloat16, tag="et", name=f"et{h}_{b}")
                pr, hh = h // 2, h % 2
                rr = l_pool.tile([1, 1024], dt.float32r, tag="rr")
                rb = rb_pool.tile([64, 1024], dt.float32, tag="rb")
                cph = [mm.tile([128, 512], dt.float32, tag=cps_tags[0], name=f"cpA{h}_{b}"),
                       mm.tile([128, 512], dt.float32, tag=cps_tags[1], name=f"cpB{h}_{b}")]
                ci = 0

                def _pv(ki):
                    g0 = ki * 128
                    while g0 < 1024:
                        half = g0 // 512
                        g1 = min(1024, (half + 1) * 512)
                        loc = slice(g0 - half * 512, g1 - half * 512)
                        nc.tensor.matmul(
                            cph[half][0:65, loc],
                            va[b][:, ki * 72: ki * 72 + 65],
                            et[:, koff[ki] + g0 - ki * 128: koff[ki] + g1 - ki * 128],
                            start=(ki == 0), stop=(ki == (3 if half == 0 else 7)),
                            skip_group_check=True,
                        )
                        g0 = g1

                def _epi(ki):
                    # epilogue for the finished half: A after ki=3, B after 7
                    half = 0 if ki == 3 else 1
                    hs_ = slice(half * 512, half * 512 + 512)
                    # reciprocal straight from the PSUM ones-row
                    # (f32r is fp32-width; the gate only knows dtype != f32)
                    with nc.allow_low_precision(reason="f32r output is fp32-width"):
                        nc.vector.reciprocal(rr[0:1, hs_], cph[half][64:65, 0:512])
                    slot = mm.tile([128, 512], dt.float32, tag="aux",
                                   name=f"rps{h}{half}_{b}")
                    nc.tensor.matmul(slot[0:64, :], ones_r[:], rr[:, hs_],
                                     start=True, stop=True)
                    nc.vector.tensor_copy(rb[:, hs_], slot[0:64, :])
                    nc.vector.tensor_mul(
                        ctxt[(b, pr)][hh * 64:(hh + 1) * 64, hs_],
                        cph[half][0:64, 0:512], rb[:, hs_])

                for ki in range(8):
                    base = ki * 128
                    nchunks = (1024 - base + 511) // 512
                    for cj in range(nchunks):
                        c0 = base + cj * 512
                        cw = min(512, 1024 - c0)
                        sc = mm.tile([128, 512], dt.float32,
                                     tag=rot[ci % len(rot)],
                                     name=f"sc{h}{ki}{cj}_{b}")
                        ci += 1
                        nc.tensor.matmul(
                            sc[:, 0:cw],
                            kk[b][0:64, base:base + 128],
                            qp[(b, h)][0:64, c0:c0 + cw],
                            start=True, stop=True,
                        )
                        abc = b * 64 + ki * 8 + h
                        nc.scalar.activation(
                            et[:, koff[ki] + (c0 - base): koff[ki] + (c0 - base) + cw],
                            sc[:, 0:cw], AF.Exp,
                            bias=ab_sb[:, abc:abc + 1], scale=INV)
                        if cj == 0:
                            # causal mask: zero the upper triangle of the diag
                            # block via a 0/1 multiply (off the sc->exp chain)
                            nc.vector.tensor_mul(
                                et[:, koff[ki]: koff[ki] + 128],
                                et[:, koff[ki]: koff[ki] + 128], msk_sb[:])
                    if pv_inline:
                        _pv(ki)
                        if ki == 3 or ki == 7:
                            _epi(ki)
                            yield
                    if ki == 3:
                        yield
                if not pv_inline:
                    yield
                    for ki in range(8):
                        _pv(ki)
                        if ki == 3 or ki == 7:
                            _epi(ki)
                            yield

        def gen_dense(b):
            # b=1 runs after attention is done, so the score-rotation banks
            # are free for deeper accumulate/drain pipelining
            slots = ("qkv0", "qkv1", "qkv2", "qkv3") if b == 0 else (
                "qkv0", "qkv1", "qkv2", "qkv3", "aux", "cpsA")
            for mt in range(32):
                dsb = dout_pool.tile([128, 1024], dt.bfloat16, tag="dsb",
                                     name=f"dsb{mt}_{b}")
                for n2 in range(2):
                    dps = mm.tile([128, 512], dt.float32,
                                  tag=slots[(mt * 2 + n2) % len(slots)],
                                  name=f"d{mt}{n2}_{b}")
                    for kt in range(4):
                        nc.tensor.matmul(
                            dps[:],
                            wd_sb[kt][:, mt * 128:(mt + 1) * 128],
                            ctxt[(b, kt)][:, n2 * 512:(n2 + 1) * 512],
                            start=(kt == 0), stop=(kt == 3),
                        )
                    if n2 == 0:
                        nc.scalar.copy(dsb[:, 0:512], dps[:])
                    else:
                        nc.vector.tensor_copy(dsb[:, 512:1024], dps[:])
                    if b == 1 and mt == 31:
                        # last tile: ship halves separately to shorten the
                        # final copy->DMA drain chain
                        nc.sync.dma_start(
                            outp_d[mt][:, b * 1024 + n2 * 512:
                                        b * 1024 + n2 * 512 + 512],
                            dsb[:, n2 * 512:(n2 + 1) * 512])
                if not (b == 1 and mt == 31):
                    eng = nc.gpsimd if mt % 2 == 0 else nc.sync
                    eng.dma_start(
                        outp_d[mt][:, b * 1024: b * 1024 + 1024], dsb[:])
                yield

        _drive(gen_qkv(0))
        _drive(gen_attn(0), gen_qkv(1))
        _drive(gen_attn(1), gen_dense(0))
        _drive(gen_dense(1))

    _CACHED_NC = nc
    return nc


def host_prep(hidden_states, alibi, attention_mask, W_qkv, W_dense):
    hsT = np.ascontiguousarray(hidden_states.reshape(NPOS, HID).T).astype(bf16)
    hsT = hsT.reshape(32, 128, NPOS)

    j32 = np.arange(32)
    inv_freq = 1.0 / (10000.0 ** (2 * j32 / HD))
    t = np.arange(S, dtype=np.float64)
    fr = np.outer(inv_freq, t)                       # [32, S]
    cst = np.zeros((128, 2048), np.float32)
    cst[:, 0:1024] = np.tile(np.cos(fr), (4, 1))
    cst[:, 1024:2048] = np.tile(np.sin(fr), (4, 1))
    cst = cst.astype(bf16)

    # single causal diag block, [kpos, q] layout: 0 where kpos > q, else 1
    mf = np.where(attention_mask[0, 0, 0:128, 0:128], 0.0, 1.0).astype(np.float32)
    msk = np.ascontiguousarray(mf.T).astype(bf16)    # [kpos, q]

    al = alibi.reshape(B, NKV * G, S) * INV          # [B, 64, S]

    perm = []
    for i in range(4):
        perm += [i * 64 + d for d in range(32)]
    for i in range(4):
        perm += [i * 64 + 32 + d for d in range(32)]
    for i in range(4, 8):
        perm += [i * 64 + d for d in range(32)]
    for i in range(4, 8):
        perm += [i * 64 + 32 + d for d in range(32)]
    perm += [512 + d for d in range(64)] + [576 + d for d in range(64)]
    perm = np.array(perm)

    idn = np.eye(64, dtype=np.float32).astype(bf16)
    in_maps = []
    for c in range(NCORES):
        Wg = W_qkv[c * 640:(c + 1) * 640][perm]       # [640, 4096]
        wq = np.ascontiguousarray(Wg.T).astype(bf16).reshape(32, 128, 640)
        Wd = W_dense[:, c * 512:(c + 1) * 512]        # [4096, 512]
        wd = np.ascontiguousarray(Wd.T).astype(bf16).reshape(4, 128, 4096)
        ab = np.zeros((128, 128), np.float32)
        for b in range(2):
            for ki in range(8):
                for h in range(8):
                    ab[:, b * 64 + ki * 8 + h] = al[b, c * 8 + h,
                                                    ki * 128:(ki + 1) * 128]
        in_maps.append({
            "hst": hsT, "wq": wq, "wd": wd, "cst": cst,
            "msk": msk, "ab": ab, "idn": idn,
        })
    return in_maps


def kernel(hidden_states, alibi, attention_mask, W_qkv, W_dense, _want_time=False):
    nc = build_program()
    in_maps = host_prep(np.asarray(hidden_states), np.asarray(alibi),
                        np.asarray(attention_mask), np.asarray(W_qkv),
                        np.asarray(W_dense))
    res = run_bass_kernel_spmd(nc, in_maps, list(range(NCORES)))
    acc = np.zeros((32, 128, NPOS), np.float32)
    for c in range(NCORES):
        acc += res.results[c]["outp"].astype(np.float32)
    out = acc.reshape(4096, NPOS).T.reshape(B, S, HID)
    if _want_time:
        return np.ascontiguousarray(out), res
    return np.ascontiguousarray(out)
